# revision 1
# baseline (speedup 1.0000x reference)
"""2-layer GAT (GATConv x2 + log_softmax) on 8 Trainium2 NeuronCores.

Strategy (dst-sharded message passing):
  - Nodes are sharded contiguously across 8 cores (6250 each); every edge is
    owned by the core owning its dst node.  Edges are grouped by dst tile
    (128 dst nodes), split into A/B halves by src id (so gather indices fit
    int16), padded to 128-edge blocks with a cross-core-uniform schedule so
    all 8 cores run one SPMD program.
  - Layer-1 node phase is replicated: every core computes h = x@W1 (bf16,
    fp32 accum) for ALL nodes and writes a gather table
    [h(512) | a_src(8) | a_dst(8) | pad] bf16 per node.
  - Edge phase per 128-edge block: dma_gather rows by src, build a 0/1
    selection matrix SelT[e,d] = (dst_local[e] == d) on DVE, per-head
    weight multiply, then PE matmul SelT.T @ M accumulates the segment sum
    (and the softmax denominator) in PSUM per dst tile.
  - Scores: exp(leaky_relu(a_src[src] + a_dst[dst])) with a_dst gathered
    from a per-core table; softmax normalization is applied per dst tile
    after aggregation (alpha = w/denom pulled out of the edge sum).
  - Layer 2 (1 head, 16 ch) reuses the same block structure; the small
    z-table is exchanged with an AllGather collective.
"""
import os
import math
import numpy as np
import ml_dtypes

import concourse.bass as bass
import concourse.mybir as mybir
import concourse.tile as tile
import concourse.bacc as bacc
from concourse.masks import make_identity
from concourse.library_config import mlp

BF = ml_dtypes.bfloat16
dt = mybir.dt
AF = mybir.ActivationFunctionType
ALU = mybir.AluOpType

P = 128
ROW1 = 640     # table1 cols (bf16): [h 512 | a_src 8 | a_dst 8 | pad]
ROWZ = 128     # z-table cols (bf16): [z 16 | a_src2 1 | a_dst2 1 | pad]
BLKCAP = 40    # max blocks per gather group


# ----------------------------------------------------------------------------
# host-side schedule construction
# ----------------------------------------------------------------------------

def _wrap_idx(vals, slots):
    """Pad `vals` with 0 to `slots`, wrap into [128, slots/16] int16 layout."""
    v = np.zeros(slots, np.int64)
    v[: len(vals)] = vals
    a = v.reshape(-1, 16).T  # [16, slots/16]
    return np.tile(a, (8, 1)).astype(np.int16)


class Schedule:
    """Cross-core-uniform block schedule + per-core index arrays."""

    def __init__(self, src, dst, n_nodes, n_cores, force_split=None):
        self.n_nodes = n_nodes
        self.n_cores = n_cores
        self.npc = n_nodes // n_cores                 # real nodes per core
        self.nt = (self.npc + P - 1) // P             # dst tiles per core
        self.npcp = self.nt * P                       # padded nodes per core
        self.ntot_p = ((n_nodes + P - 1) // P) * P if n_cores == 1 else None
        # padded global table rows (node-id indexed)
        self.table_rows = ((n_nodes + P - 1) // P) * P
        self.table_rows = max(self.table_rows, self.npcp * n_cores)
        self.zrows = self.npcp * n_cores              # z-table rows (zid indexed)

        # split for int16 gathers: src <= SPLIT1-1 -> table A half;
        # zid(src) <= 32767 must also hold.
        if force_split is not None:
            self.split1 = force_split
        elif self.table_rows <= 32768 and self.zrows <= 32768:
            self.split1 = self.table_rows  # no B half
        else:
            # largest s with s-1 <= 32767 and zid(s-1) <= 32767
            s = min(32768, self.n_nodes)
            while s > 0:
                n = s - 1
                zid = (n // self.npc) * self.npcp + (n % self.npc)
                if zid <= 32767:
                    break
                s -= 1
            self.split1 = s
        self.zsplit = ((self.split1 - 1) // self.npc) * self.npcp + (
            (self.split1 - 1) % self.npc
        ) + 1 if self.split1 < self.table_rows else self.zrows

        core = dst // self.npc
        loc = dst - core * self.npc
        t = loc // P
        dloc = loc % P
        isB = src >= self.split1

        nc_, nt_ = n_cores, self.nt
        # counts[core, tile, {A,B}]
        key = (core * nt_ + t) * 2 + isB
        cnt = np.bincount(key, minlength=nc_ * nt_ * 2).reshape(nc_, nt_, 2)
        mx = cnt.max(axis=0)                            # [nt, 2]
        self.Ablk = np.ceil(mx[:, 0] / P).astype(int)
        self.Bblk = np.ceil(mx[:, 1] / P).astype(int)
        self.TBlk = self.Ablk + self.Bblk

        # groups: consecutive tiles, sum(TBlk) <= BLKCAP
        self.groups = []
        cur, acc = [], 0
        for ti in range(nt_):
            tb = int(self.TBlk[ti])
            if cur and acc + tb > BLKCAP:
                self.groups.append(cur)
                cur, acc = [], 0
            cur.append(ti)
            acc += tb
        if cur:
            self.groups.append(cur)

        # canonical block order & per-tile positions within group buffers
        # group buffer layout: [A-blocks of each tile in order, then B-blocks]
        self.g_ablk = []   # per group: total A blocks
        self.g_tblk = []   # per group: total blocks
        self.tile_apos = {}  # tile -> in-group A block offset
        self.tile_bpos = {}  # tile -> in-group block offset of its B blocks
        self.g_base = []     # per group: global block offset
        nblocks = 0
        for g, tl in enumerate(self.groups):
            ga = int(sum(self.Ablk[ti] for ti in tl))
            gt = int(sum(self.TBlk[ti] for ti in tl))
            self.g_ablk.append(ga)
            self.g_tblk.append(gt)
            ao = 0
            bo = ga
            for ti in tl:
                self.tile_apos[ti] = ao
                self.tile_bpos[ti] = bo
                ao += int(self.Ablk[ti])
                bo += int(self.Bblk[ti])
            self.g_base.append(nblocks)
            nblocks += gt
        self.nblocks = nblocks

        # per-core arrays
        # order edges by (core, tile, isB) stably
        order = np.lexsort((isB, t, core))
        self.per_core = []
        for c in range(nc_):
            m0 = order[core[order] == c]
            idx1A_cols, idx1B_cols, idx2A_cols, idx2B_cols, idxD_cols = [], [], [], [], []
            dstloc = np.full((P, nblocks), 999.0, np.float32)
            for g, tl in enumerate(self.groups):
                a_src_l, b_src_l, d_l_A, d_l_B = [], [], [], []
                dl_A, dl_B = [], []
                for ti in tl:
                    e = m0[t[m0] == ti]
                    eA = e[~isB[e]]
                    eB = e[isB[e]]
                    nA = int(self.Ablk[ti]) * P
                    nB = int(self.Bblk[ti]) * P
                    sA = np.zeros(nA, np.int64)
                    sA[: len(eA)] = src[eA]
                    sB = np.zeros(nB, np.int64)
                    sB[: len(eB)] = src[eB] - self.split1
                    dA = np.zeros(nA, np.int64)
                    dA[: len(eA)] = dst[eA] - c * self.npc
                    dB = np.zeros(nB, np.int64)
                    dB[: len(eB)] = dst[eB] - c * self.npc
                    lA = np.full(nA, 999.0, np.float32)
                    lA[: len(eA)] = dloc[eA]
                    lB = np.full(nB, 999.0, np.float32)
                    lB[: len(eB)] = dloc[eB]
                    a_src_l.append(sA)
                    b_src_l.append(sB)
                    d_l_A.append(dA)
                    d_l_B.append(dB)
                    dl_A.append(lA)
                    dl_B.append(lB)
                gsA = np.concatenate(a_src_l) if a_src_l else np.zeros(0, np.int64)
                gsB = np.concatenate(b_src_l) if b_src_l else np.zeros(0, np.int64)
                gdl = np.concatenate(dl_A + dl_B) if (dl_A or dl_B) else np.zeros(0, np.float32)
                gd = np.concatenate(d_l_A + d_l_B) if (d_l_A or d_l_B) else np.zeros(0, np.int64)
                # L2 indices: zid mapping of global src
                def zid_of(v):
                    vv = np.asarray(v, np.int64)
                    return (vv // self.npc) * self.npcp + (vv % self.npc)
                g2A = zid_of(gsA)                       # gsA holds global src (pads=0)
                g2B = zid_of(gsB + self.split1) - self.zsplit
                idx1A_cols.append(_wrap_idx(gsA, len(gsA)))
                idx1B_cols.append(_wrap_idx(gsB, len(gsB)))
                idx2A_cols.append(_wrap_idx(g2A, len(g2A)))
                idx2B_cols.append(_wrap_idx(g2B, len(g2B)))
                idxD_cols.append(_wrap_idx(gd, len(gd)))
                gb = self.g_base[g]
                dstloc[:, gb : gb + self.g_tblk[g]] = gdl.reshape(-1, P).T
            cat = lambda ls: (
                np.concatenate(ls, axis=1) if ls and sum(x.shape[1] for x in ls) else np.zeros((P, 1), np.int16)
            )
            self.per_core.append(
                dict(
                    idx1A=cat(idx1A_cols), idx1B=cat(idx1B_cols),
                    idx2A=cat(idx2A_cols), idx2B=cat(idx2B_cols),
                    idxD=cat(idxD_cols), dstloc=dstloc,
                )
            )
        # column offsets per group in the concatenated idx arrays
        self.gA_coloff, self.gB_coloff, self.gD_coloff = [], [], []
        a = b = d = 0
        for g in range(len(self.groups)):
            self.gA_coloff.append(a)
            self.gB_coloff.append(b)
            self.gD_coloff.append(d)
            a += (self.g_ablk[g] * P) // 16
            b += ((self.g_tblk[g] - self.g_ablk[g]) * P) // 16
            d += (self.g_tblk[g] * P) // 16
        self.totA_cols = max(a, 1)
        self.totB_cols = max(b, 1)
        self.totD_cols = max(d, 1)


# ----------------------------------------------------------------------------
# device program
# ----------------------------------------------------------------------------

def build_program(sched: Schedule, n_cores: int, phase: str = 'full'):
    """Build the SPMD Bass/Tile program for the given schedule."""
    nc = bacc.Bacc(None, target_bir_lowering=False, debug=True, num_devices=n_cores)

    TR = sched.table_rows
    ZR = sched.zrows
    NT = sched.nt
    NPC, NPCP = sched.npc, sched.npcp
    NODE_TILES = TR // P

    # ---- inputs -------------------------------------------------------------
    xT = nc.dram_tensor("xT", [P, 2, TR], dt.bfloat16, kind="ExternalInput")
    W1r = nc.dram_tensor("W1r", [P, 2, 512], dt.bfloat16, kind="ExternalInput")
    W1Tr = nc.dram_tensor("W1Tr", [P, 4, 256], dt.bfloat16, kind="ExternalInput")
    Acat = nc.dram_tensor("Acat", [P, 4, 16], dt.bfloat16, kind="ExternalInput")
    W2r = nc.dram_tensor("W2r", [P, 4, 16], dt.bfloat16, kind="ExternalInput")
    W2Tr = nc.dram_tensor("W2Tr", [16, 512], dt.bfloat16, kind="ExternalInput")
    att2 = nc.dram_tensor("att2", [16, 2], dt.bfloat16, kind="ExternalInput")
    idx1A = nc.dram_tensor("idx1A", [P, sched.totA_cols], dt.int16, kind="ExternalInput")
    idx1B = nc.dram_tensor("idx1B", [P, sched.totB_cols], dt.int16, kind="ExternalInput")
    idx2A = nc.dram_tensor("idx2A", [P, sched.totA_cols], dt.int16, kind="ExternalInput")
    idx2B = nc.dram_tensor("idx2B", [P, sched.totB_cols], dt.int16, kind="ExternalInput")
    idxD = nc.dram_tensor("idxD", [P, sched.totD_cols], dt.int16, kind="ExternalInput")
    dstlocr = nc.dram_tensor("dstloc", [P, sched.nblocks], dt.float32, kind="ExternalInput")
    out_shard = nc.dram_tensor("out_shard", [NPCP, 16], dt.float32, kind="ExternalOutput")

    with tile.TileContext(nc) as tc:
        nc.gpsimd.load_library(mlp)
        with (
            tc.tile_pool(name="dram", bufs=1, space="DRAM") as dram,
            tc.tile_pool(name="const", bufs=1) as cpool,
        ):
            table1 = dram.tile([TR, ROW1], dt.bfloat16)
            adst_own = dram.tile([NPCP, ROWZ], dt.bfloat16)
            cc_in = dram.tile([NPCP, ROWZ], dt.bfloat16)
            cc_out = dram.tile([ZR, ROWZ], dt.bfloat16,
                               addr_space=("Shared" if n_cores > 1 else "Local"))

            # ---- constants -------------------------------------------------
            iota_i = cpool.tile([P, P], dt.int32)
            nc.gpsimd.iota(iota_i[:], pattern=[[1, P]], base=0, channel_multiplier=0)
            iota_bf = cpool.tile([P, P], dt.bfloat16)
            nc.vector.tensor_copy(iota_bf[:], iota_i[:])
            ident = cpool.tile([P, P], dt.bfloat16)
            make_identity(nc, ident[:])

            W1s = cpool.tile([P, 2, 512], dt.bfloat16)
            nc.sync.dma_start(W1s[:], W1r[:])
            W1Ts = cpool.tile([P, 4, 256], dt.bfloat16)
            nc.sync.dma_start(W1Ts[:], W1Tr[:])
            Acats = cpool.tile([P, 4, 16], dt.bfloat16)
            nc.sync.dma_start(Acats[:], Acat[:])
            W2Ts = cpool.tile([16, 512], dt.bfloat16)
            nc.sync.dma_start(W2Ts[:], W2Tr[:])
            att2s = cpool.tile([16, 2], dt.bfloat16)
            nc.sync.dma_start(att2s[:], att2[:])
            dstloc_s = cpool.tile([P, sched.nblocks], dt.float32)
            nc.sync.dma_start(dstloc_s[:], dstlocr[:])
            i1A = cpool.tile([P, sched.totA_cols], dt.int16)
            nc.sync.dma_start(i1A[:], idx1A[:])
            i1B = cpool.tile([P, sched.totB_cols], dt.int16)
            nc.sync.dma_start(i1B[:], idx1B[:])
            i2A = cpool.tile([P, sched.totA_cols], dt.int16)
            nc.sync.dma_start(i2A[:], idx2A[:])
            i2B = cpool.tile([P, sched.totB_cols], dt.int16)
            nc.sync.dma_start(i2B[:], idx2B[:])
            iD = cpool.tile([P, sched.totD_cols], dt.int16)
            nc.sync.dma_start(iD[:], idxD[:])

            # fused attention weights: wcat = W1 @ [Asrc|Adst]  -> [2,128,16]
            wcat = cpool.tile([P, 2, 16], dt.bfloat16)
            wz = cpool.tile([P, 4, 18], dt.bfloat16)   # [W2 | wcat2] per 128-chunk
            W2s = cpool.tile([P, 4, 16], dt.bfloat16)
            nc.sync.dma_start(W2s[:], W2r[:])
            with tc.tile_pool(name="p0ps", bufs=2, space="PSUM") as p0ps:
                for i in range(2):
                    ps = p0ps.tile([P, 16], dt.float32, tag="wc")
                    for j in range(4):
                        nc.tensor.matmul(
                            ps[:], W1Ts[:, j, i * P:(i + 1) * P], Acats[:, j, :],
                            start=(j == 0), stop=(j == 3),
                        )
                    nc.vector.tensor_copy(wcat[:, i, :], ps[:])
                for cch in range(4):
                    ps2 = p0ps.tile([P, 2], dt.float32, tag="wz")
                    nc.tensor.matmul(
                        ps2[:], W2Ts[:, cch * P:(cch + 1) * P], att2s[:],
                        start=True, stop=True,
                    )
                    nc.vector.tensor_copy(wz[:, cch, 16:18], ps2[:])
                    nc.vector.tensor_copy(wz[:, cch, 0:16], W2s[:, cch, :])

            # ---- P1: replicated node phase --------------------------------
            XB = 4  # node tiles per x load
            with (
                tc.tile_pool(name="p1sb", bufs=3) as p1sb,
                tc.tile_pool(name="p1ps", bufs=2, space="PSUM") as p1ps,
            ):
                for tq in range(0, NODE_TILES, XB):
                    nb = min(XB, NODE_TILES - tq)
                    xt = p1sb.tile([P, 2, nb * P], dt.bfloat16, tag="xt")
                    nc.sync.dma_start(xt[:], xT[:, :, tq * P: tq * P + nb * P])
                    for u in range(nb):
                        ph = p1ps.tile([P, 512], dt.float32, tag="ph")
                        pa = p1ps.tile([P, 16], dt.float32, tag="pa")
                        for c in range(2):
                            lhs = xt[:, c, u * P:(u + 1) * P]
                            nc.tensor.matmul(ph[:], lhs, W1s[:, c, :], start=(c == 0), stop=(c == 1))
                            nc.tensor.matmul(pa[:], lhs, wcat[:, c, :], start=(c == 0), stop=(c == 1))
                        rowt = p1sb.tile([P, ROW1], dt.bfloat16, tag="rowt")
                        nc.scalar.copy(rowt[:, 0:512], ph[:])
                        nc.vector.tensor_copy(rowt[:, 512:528], pa[:])
                        nc.vector.memset(rowt[:, 528:ROW1], 0.0)
                        nc.sync.dma_start(
                            table1[(tq + u) * P:(tq + u + 1) * P, :], rowt[:]
                        )

            # ---- P1.5: per-core a_dst table -------------------------------
            if phase not in ("p1",):
                rbase = nc.sync.partition_id() * NPC
                nc.sync.dma_start(
                    adst_own[:, :],
                    table1[bass.ds(rbase, NPCP), 512:512 + ROWZ],
                )

            # ---- edge phase helper ----------------------------------------
            def edge_phase(layer):
                """layer 1: table1 gathers, 8 heads; layer 2: z-table, 1 head."""
                sub = os.environ.get("GAT_L1SUB", "full")
                if layer == 1:
                    g_src_tab_A, g_src_tab_B = table1, table1[sched.split1:, :]
                    g_elem, g_row = ROW1, ROW1
                    iA, iB = i1A, i1B
                    NH = 8
                else:
                    g_src_tab_A, g_src_tab_B = cc_out, cc_out[sched.zsplit:, :]
                    g_elem, g_row = ROWZ, ROWZ
                    iA, iB = i2A, i2B
                    NH = 1
                with (
                    tc.tile_pool(name=f"ed{layer}", bufs=2) as ep,
                    tc.tile_pool(name=f"eb{layer}", bufs=3) as bp,
                    tc.tile_pool(name=f"ep{layer}", bufs=2, space="PSUM") as pp,
                ):
                    for g, tl in enumerate(sched.groups):
                        GB = sched.g_tblk[g]
                        GA = sched.g_ablk[g]
                        nA, nB_ = GA * P, (GB - GA) * P
                        hg = ep.tile([P, GB, g_elem], dt.bfloat16, tag="hg")
                        if nA:
                            nc.gpsimd.dma_gather(
                                hg[:, 0:GA, :], g_src_tab_A[:],
                                iA[:, sched.gA_coloff[g]: sched.gA_coloff[g] + nA // 16],
                                nA, nA, g_elem, single_packet=False,
                            )
                        if nB_:
                            nc.gpsimd.dma_gather(
                                hg[:, GA:GB, :], g_src_tab_B,
                                iB[:, sched.gB_coloff[g]: sched.gB_coloff[g] + nB_ // 16],
                                nB_, nB_, g_elem, elem_step=g_row,
                                single_packet=False,
                            )
                        aD = ep.tile([P, GB, ROWZ], dt.bfloat16, tag="aD")
                        dtab = adst_own if layer == 1 else cc_in
                        nc.gpsimd.dma_gather(
                            aD[:], dtab[:],
                            iD[:, sched.gD_coloff[g]: sched.gD_coloff[g] + (GB * P) // 16],
                            GB * P, GB * P, ROWZ, single_packet=False,
                        )
                        if sub == "gather":
                            continue
                        for ti in tl:
                            ab, bb = int(sched.Ablk[ti]), int(sched.Bblk[ti])
                            tb = ab + bb
                            if tb == 0:
                                continue
                            apos, bpos = sched.tile_apos[ti], sched.tile_bpos[ti]
                            gb = sched.g_base[g]
                            # (in-buffer block, w-slot) pairs for this tile
                            spans = []
                            if ab:
                                spans.append((apos, ab, 0))
                            if bb:
                                spans.append((bpos, bb, ab))
                            wt = bp.tile([P, tb * NH], dt.float32, tag="wt")
                            wbf = bp.tile([P, tb * NH], dt.bfloat16, tag="wbf")
                            for (o, n, wo) in spans:
                                st = bp.tile([P, n * NH], dt.float32, tag="st")
                                if layer == 1:
                                    a_s = hg[:, o:o + n, 512:520]
                                    a_d = aD[:, o:o + n, 8:16]
                                else:
                                    a_s = hg[:, o:o + n, 16:17]
                                    a_d = aD[:, o:o + n, 17:18]
                                st3 = st[:].rearrange("p (n k) -> p n k", k=NH)
                                nc.vector.tensor_tensor(
                                    out=st3, in0=a_s, in1=a_d, op=ALU.add
                                )
                                lk = bp.tile([P, n * NH], dt.float32, tag="lk")
                                nc.vector.scalar_tensor_tensor(
                                    lk[:], st[:], 0.2, st[:], ALU.mult, ALU.max
                                )
                                nc.scalar.activation(
                                    wt[:, wo * NH:(wo + n) * NH], lk[:], AF.Exp
                                )
                            nc.vector.tensor_copy(wbf[:], wt[:])
                            if sub == "score":
                                continue
                            psum_o = pp.tile([P, NH * (64 if layer == 1 else 16)], dt.float32, tag="psO")
                            psum_d = pp.tile([P, NH], dt.float32, tag="psD")
                            j = 0
                            for (o, n, wo) in spans:
                                for k in range(n):
                                    blk = o + k
                                    gcol = gb + blk
                                    selT = bp.tile([P, P], dt.bfloat16, tag="selT")
                                    nc.vector.tensor_scalar(
                                        out=selT[:], in0=iota_bf[:],
                                        scalar1=dstloc_s[:, gcol:gcol + 1],
                                        scalar2=None, op0=ALU.is_equal,
                                    )
                                    first, last = (j == 0), (j == tb - 1)
                                    ws = (wo + k) * NH
                                    if layer == 1:
                                        M = bp.tile([P, 512], dt.bfloat16, tag="M")
                                        wbc = wbf[:, ws:ws + 8].rearrange(
                                            "p (a b) -> p a b", b=1
                                        ).to_broadcast([P, 8, 64])
                                        nc.vector.tensor_tensor(
                                            out=M[:].rearrange("p (a b) -> p a b", b=64),
                                            in0=hg[:, blk, 0:512].rearrange("p (a b) -> p a b", b=64),
                                            in1=wbc,
                                            op=ALU.mult,
                                        )
                                    else:
                                        M = bp.tile([P, 16], dt.bfloat16, tag="M")
                                        nc.vector.tensor_scalar_mul(
                                            M[:], hg[:, blk, 0:16], wt[:, ws:ws + 1]
                                        )
                                    nc.tensor.matmul(psum_o[:], selT[:], M[:], start=first, stop=last)
                                    nc.tensor.matmul(
                                        psum_d[:], selT[:], wbf[:, ws:ws + NH],
                                        start=first, stop=last,
                                    )
                                    j += 1
                            if sub == "blocks":
                                continue
                            close_tile(layer, ti, psum_o, psum_d, bp, pp)

            # ---- tile close -------------------------------------------------
            def close_tile(layer, ti, psum_o, psum_d, bp, pp):
                if layer == 1:
                    r = bp.tile([P, 8], dt.float32, tag="r")
                    nc.vector.reciprocal(r[:], psum_d[:])
                    o1 = bp.tile([P, 512], dt.bfloat16, tag="o1")
                    for h in range(8):
                        sl = slice(h * 64, (h + 1) * 64)
                        if h % 2 == 0:
                            nc.scalar.activation(
                                o1[:, sl], psum_o[:, sl], AF.Copy, scale=r[:, h:h + 1]
                            )
                        else:
                            nc.vector.tensor_scalar_mul(o1[:, sl], psum_o[:, sl], r[:, h:h + 1])
                    # elu: h2 = max(o1,0) + exp(min(o1,0)) - 1
                    u = bp.tile([P, 512], dt.bfloat16, tag="u")
                    nc.vector.tensor_scalar_min(u[:], o1[:], 0.0)
                    e1 = bp.tile([P, 512], dt.bfloat16, tag="e1")
                    nc.scalar.activation(e1[:], u[:], AF.Exp)
                    rv = bp.tile([P, 512], dt.bfloat16, tag="rv")
                    nc.vector.tensor_scalar_max(rv[:], o1[:], 0.0)
                    h2 = bp.tile([P, 512], dt.bfloat16, tag="h2")
                    nc.vector.scalar_tensor_tensor(
                        h2[:], e1[:], -1.0, rv[:], ALU.add, ALU.add
                    )
                    # transpose h2 -> z matmuls
                    pz = pp.tile([P, 18], dt.float32, tag="psZ")
                    for c in range(4):
                        ptr = pp.tile([P, P], dt.bfloat16, tag="psT")
                        nc.tensor.transpose(ptr[:], h2[:, c * P:(c + 1) * P], ident[:])
                        h2T = bp.tile([P, P], dt.bfloat16, tag="h2T")
                        nc.scalar.copy(h2T[:], ptr[:])
                        nc.tensor.matmul(pz[:], h2T[:], wz[:, c, :], start=(c == 0), stop=(c == 3))
                    zrow = bp.tile([P, ROWZ], dt.bfloat16, tag="zrow")
                    nc.vector.tensor_copy(zrow[:, 0:18], pz[:])
                    nc.vector.memset(zrow[:, 18:ROWZ], 0.0)
                    nc.sync.dma_start(cc_in[ti * P:(ti + 1) * P, :], zrow[:])
                else:
                    r2 = bp.tile([P, 1], dt.float32, tag="r2")
                    nc.vector.reciprocal(r2[:], psum_d[:])
                    o2 = bp.tile([P, 16], dt.float32, tag="o2")
                    nc.vector.tensor_scalar_mul(o2[:], psum_o[:], r2[:, 0:1])
                    mx = bp.tile([P, 1], dt.float32, tag="mx")
                    nc.vector.tensor_reduce(mx[:], o2[:], axis=mybir.AxisListType.X, op=ALU.max)
                    o2m = bp.tile([P, 16], dt.float32, tag="o2m")
                    nc.vector.tensor_scalar_sub(o2m[:], o2[:], mx[:, 0:1])
                    ex = bp.tile([P, 16], dt.float32, tag="ex")
                    ssum = bp.tile([P, 1], dt.float32, tag="ssum")
                    nc.scalar.activation(ex[:], o2m[:], AF.Exp, accum_out=ssum[:])
                    lse = bp.tile([P, 1], dt.float32, tag="lse")
                    nc.scalar.activation(lse[:], ssum[:], AF.Ln)
                    res = bp.tile([P, 16], dt.float32, tag="res")
                    nc.vector.tensor_scalar_sub(res[:], o2m[:], lse[:, 0:1])
                    nc.sync.dma_start(out_shard[ti * P:(ti + 1) * P, :], res[:])

            if phase not in ("p1", "p15"):
                edge_phase(1)

            if phase in ("cc", "full"):
                # ---- z-table exchange -------------------------------------
                if n_cores == 1:
                    nc.sync.dma_start(cc_out[:, :], cc_in[:, :])
                else:
                    nc.gpsimd.collective_compute(
                        "AllGather", ALU.bypass,
                        ins=[cc_in[:]], outs=[cc_out[:]],
                        replica_groups=[list(range(n_cores))],
                    )

            if phase == "full":
                edge_phase(2)

    nc.compile()
    return nc


# ----------------------------------------------------------------------------
# host entry
# ----------------------------------------------------------------------------

def _blockdiag(att, heads, hid):
    """[heads, hid] -> [heads*hid, heads] block diagonal."""
    out = np.zeros((heads * hid, max(heads, 1)), np.float32)
    for h in range(heads):
        out[h * hid:(h + 1) * hid, h] = att[h]
    return out


def prepare_inputs(inputs, sched: Schedule):
    x = np.asarray(inputs["x"], np.float32)
    ei = np.asarray(inputs["edge_index"])
    W1 = np.asarray(inputs["W1"], np.float32)
    as1 = np.asarray(inputs["att_src1"], np.float32)
    ad1 = np.asarray(inputs["att_dst1"], np.float32)
    W2 = np.asarray(inputs["W2"], np.float32)
    as2 = np.asarray(inputs["att_src2"], np.float32)
    ad2 = np.asarray(inputs["att_dst2"], np.float32)

    N, IN = x.shape
    TR = sched.table_rows
    xp = np.zeros((TR, IN), np.float32)
    xp[:N] = x
    xTb = np.ascontiguousarray(
        xp.T.reshape(2, P, TR).transpose(1, 0, 2)).astype(BF)
    W1b = np.ascontiguousarray(W1.reshape(2, P, 512).transpose(1, 0, 2)).astype(BF)
    W1Tb = np.ascontiguousarray(
        W1.T.reshape(4, P, 256).transpose(1, 0, 2)).astype(BF)
    AcatB = np.ascontiguousarray(
        np.concatenate([_blockdiag(as1, 8, 64), _blockdiag(ad1, 8, 64)], axis=1)
        .reshape(4, P, 16).transpose(1, 0, 2)
    ).astype(BF)
    W2b = np.ascontiguousarray(W2.reshape(4, P, 16).transpose(1, 0, 2)).astype(BF)
    W2Tb = np.ascontiguousarray(W2.T).astype(BF)          # [16, 512]
    att2b = np.concatenate([as2.T, ad2.T], axis=1).astype(BF)  # [16, 2]

    shared = dict(xT=xTb, W1r=W1b, W1Tr=W1Tb, Acat=AcatB, W2r=W2b, W2Tr=W2Tb, att2=att2b)
    maps = []
    for c in range(sched.n_cores):
        pc = sched.per_core[c]
        m = dict(shared)
        m.update(
            idx1A=pc["idx1A"], idx1B=pc["idx1B"], idx2A=pc["idx2A"],
            idx2B=pc["idx2B"], idxD=pc["idxD"], dstloc=pc["dstloc"],
        )
        maps.append(m)
    return maps


_LAST_RESULT = {}


def kernel(**inputs):
    from concourse.bass_utils import run_bass_kernel_spmd

    x = np.asarray(inputs["x"], np.float32)
    ei = np.asarray(inputs["edge_index"], np.int64)
    N = x.shape[0]
    n_cores = 8
    loops = np.arange(N, dtype=np.int64)
    src = np.concatenate([ei[0], loops])
    dst = np.concatenate([ei[1], loops])

    sched = Schedule(src, dst, N, n_cores)
    phase = os.environ.get("GAT_PHASE", "full")
    nc = build_program(sched, n_cores, phase=phase)
    in_maps = prepare_inputs(inputs, sched)

    trace = bool(int(os.environ.get("GAT_TRACE", "0")))
    res = run_bass_kernel_spmd(
        nc, in_maps, core_ids=list(range(n_cores)), trace=trace,
    )
    _LAST_RESULT["res"] = res

    out = np.zeros((N, 16), np.float32)
    for c in range(n_cores):
        sh = res.results[c]["out_shard"]
        n0 = c * sched.npc
        out[n0:n0 + sched.npc] = sh[: sched.npc]
    return out



# revision 13
# speedup vs baseline: 1.0329x; 1.0329x over previous
"""2-layer GAT (GATConv x2 + log_softmax) on 8 Trainium2 NeuronCores.

Strategy (dst-sharded message passing):
  - Nodes are sharded contiguously across 8 cores (6250 each); every edge is
    owned by the core owning its dst node.  Edges are grouped by dst tile
    (128 dst nodes), split into A/B halves by src id (so gather indices fit
    int16), padded to 128-edge blocks with a cross-core-uniform schedule so
    all 8 cores run one SPMD program.
  - Layer-1 node phase is replicated: every core computes h = x@W1 (bf16,
    fp32 accum) for ALL nodes and writes a gather table
    [h(512) | a_src(8) | a_dst(8) | pad] bf16 per node.
  - Edge phase per 128-edge block: dma_gather rows by src, build a 0/1
    selection matrix SelT[e,d] = (dst_local[e] == d) on DVE, per-head
    weight multiply, then PE matmul SelT.T @ M accumulates the segment sum
    (and the softmax denominator) in PSUM per dst tile.
  - Scores: exp(leaky_relu(a_src[src] + a_dst[dst])) with a_dst gathered
    from a per-core table; softmax normalization is applied per dst tile
    after aggregation (alpha = w/denom pulled out of the edge sum).
  - Layer 2 (1 head, 16 ch) reuses the same block structure; the small
    z-table is exchanged with an AllGather collective.
"""
import os
import math
import numpy as np
import ml_dtypes

import concourse.bass as bass
import concourse.mybir as mybir
import concourse.tile as tile
import concourse.bacc as bacc
from concourse.masks import make_identity
from concourse.library_config import mlp

BF = ml_dtypes.bfloat16
dt = mybir.dt
AF = mybir.ActivationFunctionType
ALU = mybir.AluOpType

P = 128
ROW1 = 640     # table1 cols (bf16): [h 512 | a_src 8 | a_dst 8 | pad]
ROWZ = 128     # z-table cols (bf16): [z 16 | a_src2 1 | a_dst2 1 | pad]
BLKCAP = 40    # max blocks per gather group


# ----------------------------------------------------------------------------
# host-side schedule construction
# ----------------------------------------------------------------------------

def _wrap_idx(vals, slots):
    """Pad `vals` with 0 to `slots`, wrap into [128, slots/16] int16 layout."""
    v = np.zeros(slots, np.int64)
    v[: len(vals)] = vals
    a = v.reshape(-1, 16).T  # [16, slots/16]
    return np.tile(a, (8, 1)).astype(np.int16)


class Schedule:
    """Cross-core-uniform block schedule + per-core index arrays."""

    def __init__(self, src, dst, n_nodes, n_cores, force_split=None):
        self.n_nodes = n_nodes
        self.n_cores = n_cores
        self.npc = n_nodes // n_cores                 # real nodes per core
        self.nt = (self.npc + P - 1) // P             # dst tiles per core
        self.npcp = self.nt * P                       # padded nodes per core
        self.ntot_p = ((n_nodes + P - 1) // P) * P if n_cores == 1 else None
        # padded global table rows (node-id indexed)
        self.table_rows = ((n_nodes + P - 1) // P) * P
        self.table_rows = max(self.table_rows, self.npcp * n_cores)
        self.zrows = self.npcp * n_cores              # z-table rows (zid indexed)

        # split for int16 gathers: src <= SPLIT1-1 -> table A half;
        # zid(src) <= 32767 must also hold.
        if force_split is not None:
            self.split1 = force_split
        elif self.table_rows <= 32768 and self.zrows <= 32768:
            self.split1 = self.table_rows  # no B half
        else:
            # largest s with s-1 <= 32767 and zid(s-1) <= 32767
            s = min(32768, self.n_nodes)
            while s > 0:
                n = s - 1
                zid = (n // self.npc) * self.npcp + (n % self.npc)
                if zid <= 32767:
                    break
                s -= 1
            self.split1 = s
        self.zsplit = ((self.split1 - 1) // self.npc) * self.npcp + (
            (self.split1 - 1) % self.npc
        ) + 1 if self.split1 < self.table_rows else self.zrows

        core = dst // self.npc
        loc = dst - core * self.npc
        t = loc // P
        dloc = loc % P
        isB = src >= self.split1

        nc_, nt_ = n_cores, self.nt
        # counts[core, tile, {A,B}]
        key = (core * nt_ + t) * 2 + isB
        cnt = np.bincount(key, minlength=nc_ * nt_ * 2).reshape(nc_, nt_, 2)
        mx = cnt.max(axis=0)                            # [nt, 2]
        self.Ablk = np.ceil(mx[:, 0] / P).astype(int)
        self.Bblk = np.ceil(mx[:, 1] / P).astype(int)
        self.TBlk = self.Ablk + self.Bblk

        # groups: consecutive tiles, sum(TBlk) <= BLKCAP
        self.groups = []
        cur, acc = [], 0
        for ti in range(nt_):
            tb = int(self.TBlk[ti])
            if cur and acc + tb > BLKCAP:
                self.groups.append(cur)
                cur, acc = [], 0
            cur.append(ti)
            acc += tb
        if cur:
            self.groups.append(cur)

        # canonical block order & per-tile positions within group buffers
        # group buffer layout: [A-blocks of each tile in order, then B-blocks]
        self.g_ablk = []   # per group: total A blocks
        self.g_tblk = []   # per group: total blocks
        self.tile_apos = {}  # tile -> in-group A block offset
        self.tile_bpos = {}  # tile -> in-group block offset of its B blocks
        self.g_base = []     # per group: global block offset
        nblocks = 0
        for g, tl in enumerate(self.groups):
            ga = int(sum(self.Ablk[ti] for ti in tl))
            gt = int(sum(self.TBlk[ti] for ti in tl))
            self.g_ablk.append(ga)
            self.g_tblk.append(gt)
            ao = 0
            bo = ga
            for ti in tl:
                self.tile_apos[ti] = ao
                self.tile_bpos[ti] = bo
                ao += int(self.Ablk[ti])
                bo += int(self.Bblk[ti])
            self.g_base.append(nblocks)
            nblocks += gt
        self.nblocks = nblocks

        self.maxtb = int(self.TBlk.max())

        # per-core arrays
        # order edges by (core, tile, isB) stably
        order = np.lexsort((isB, t, core))
        self.per_core = []
        for c in range(nc_):
            m0 = order[core[order] == c]
            idx1A_cols, idx1B_cols, idx2A_cols, idx2B_cols = [], [], [], []
            dstloc = np.full((P, nblocks), 999.0, np.float32)
            for g, tl in enumerate(self.groups):
                a_src_l, b_src_l = [], []
                dl_A, dl_B = [], []
                for ti in tl:
                    e = m0[t[m0] == ti]
                    eA = e[~isB[e]]
                    eB = e[isB[e]]
                    nA = int(self.Ablk[ti]) * P
                    nB = int(self.Bblk[ti]) * P
                    sA = np.zeros(nA, np.int64)
                    sA[: len(eA)] = src[eA]
                    sB = np.zeros(nB, np.int64)
                    sB[: len(eB)] = src[eB] - self.split1
                    lA = np.full(nA, 999.0, np.float32)
                    lA[: len(eA)] = dloc[eA]
                    lB = np.full(nB, 999.0, np.float32)
                    lB[: len(eB)] = dloc[eB]
                    a_src_l.append(sA)
                    b_src_l.append(sB)
                    dl_A.append(lA)
                    dl_B.append(lB)
                gsA = np.concatenate(a_src_l) if a_src_l else np.zeros(0, np.int64)
                gsB = np.concatenate(b_src_l) if b_src_l else np.zeros(0, np.int64)
                gdl = np.concatenate(dl_A + dl_B) if (dl_A or dl_B) else np.zeros(0, np.float32)
                # L2 indices: zid mapping of global src
                def zid_of(v):
                    vv = np.asarray(v, np.int64)
                    return (vv // self.npc) * self.npcp + (vv % self.npc)
                g2A = zid_of(gsA)                       # gsA holds global src (pads=0)
                g2B = zid_of(gsB + self.split1) - self.zsplit
                idx1A_cols.append(_wrap_idx(gsA, len(gsA)))
                idx1B_cols.append(_wrap_idx(gsB, len(gsB)))
                idx2A_cols.append(_wrap_idx(g2A, len(g2A)))
                idx2B_cols.append(_wrap_idx(g2B, len(g2B)))
                gb = self.g_base[g]
                dstloc[:, gb : gb + self.g_tblk[g]] = gdl.reshape(-1, P).T
            cat = lambda ls: (
                np.concatenate(ls, axis=1) if ls and sum(x.shape[1] for x in ls) else np.zeros((P, 1), np.int16)
            )
            self.per_core.append(
                dict(
                    idx1A=cat(idx1A_cols), idx1B=cat(idx1B_cols),
                    idx2A=cat(idx2A_cols), idx2B=cat(idx2B_cols),
                    dstloc=dstloc,
                )
            )
        # column offsets per group in the concatenated idx arrays
        self.gA_coloff, self.gB_coloff = [], []
        a = b = 0
        for g in range(len(self.groups)):
            self.gA_coloff.append(a)
            self.gB_coloff.append(b)
            a += (self.g_ablk[g] * P) // 16
            b += ((self.g_tblk[g] - self.g_ablk[g]) * P) // 16
        self.totA_cols = max(a, 1)
        self.totB_cols = max(b, 1)


# ----------------------------------------------------------------------------
# device program
# ----------------------------------------------------------------------------

def build_program(sched: Schedule, n_cores: int, phase: str = 'full'):
    """Build the SPMD Bass/Tile program for the given schedule."""
    nc = bacc.Bacc(None, target_bir_lowering=False, debug=True, num_devices=n_cores)

    TR = sched.table_rows
    ZR = sched.zrows
    NT = sched.nt
    NPC, NPCP = sched.npc, sched.npcp
    NODE_TILES = TR // P

    # ---- inputs -------------------------------------------------------------
    xT = nc.dram_tensor("xT", [P, 2, TR], dt.bfloat16, kind="ExternalInput")
    W1r = nc.dram_tensor("W1r", [P, 2, 512], dt.bfloat16, kind="ExternalInput")
    W1Tr = nc.dram_tensor("W1Tr", [P, 4, 256], dt.bfloat16, kind="ExternalInput")
    Acat = nc.dram_tensor("Acat", [P, 4, 16], dt.bfloat16, kind="ExternalInput")
    W2r = nc.dram_tensor("W2r", [P, 4, 16], dt.bfloat16, kind="ExternalInput")
    W2Tr = nc.dram_tensor("W2Tr", [16, 512], dt.bfloat16, kind="ExternalInput")
    att2 = nc.dram_tensor("att2", [16, 2], dt.bfloat16, kind="ExternalInput")
    idx1A = nc.dram_tensor("idx1A", [P, sched.totA_cols], dt.int16, kind="ExternalInput")
    idx1B = nc.dram_tensor("idx1B", [P, sched.totB_cols], dt.int16, kind="ExternalInput")
    idx2A = nc.dram_tensor("idx2A", [P, sched.totA_cols], dt.int16, kind="ExternalInput")
    idx2B = nc.dram_tensor("idx2B", [P, sched.totB_cols], dt.int16, kind="ExternalInput")
    dstlocr = nc.dram_tensor("dstloc", [P, sched.nblocks], dt.float32, kind="ExternalInput")
    out_shard = nc.dram_tensor("out_shard", [NPCP, 16], dt.float32, kind="ExternalOutput")

    with tile.TileContext(nc) as tc:
        nc.gpsimd.load_library(mlp)
        with (
            tc.tile_pool(name="dram", bufs=1, space="DRAM") as dram,
            tc.tile_pool(name="const", bufs=1) as cpool,
        ):
            table1 = dram.tile([TR, ROW1], dt.bfloat16)
            adst_own = dram.tile([NPCP, 8], dt.bfloat16)
            cc_in = dram.tile([NPCP, ROWZ], dt.bfloat16)
            cc_out = dram.tile([ZR, ROWZ], dt.bfloat16,
                               addr_space=("Shared" if n_cores > 1 else "Local"))

            # ---- constants -------------------------------------------------
            iota_i = cpool.tile([P, P], dt.int32)
            nc.gpsimd.iota(iota_i[:], pattern=[[1, P]], base=0, channel_multiplier=0)
            iota_bf = cpool.tile([P, P], dt.bfloat16)
            nc.vector.tensor_copy(iota_bf[:], iota_i[:])
            ident = cpool.tile([P, P], dt.bfloat16)
            make_identity(nc, ident[:])

            W1s = cpool.tile([P, 2, 512], dt.bfloat16)
            nc.sync.dma_start(W1s[:], W1r[:])
            W1Ts = cpool.tile([P, 4, 256], dt.bfloat16)
            nc.sync.dma_start(W1Ts[:], W1Tr[:])
            Acats = cpool.tile([P, 4, 16], dt.bfloat16)
            nc.sync.dma_start(Acats[:], Acat[:])
            W2Ts = cpool.tile([16, 512], dt.bfloat16)
            nc.sync.dma_start(W2Ts[:], W2Tr[:])
            att2s = cpool.tile([16, 2], dt.bfloat16)
            nc.sync.dma_start(att2s[:], att2[:])
            dstloc_s = cpool.tile([P, sched.nblocks], dt.float32)
            nc.sync.dma_start(dstloc_s[:], dstlocr[:])
            i1A = cpool.tile([P, sched.totA_cols], dt.int16)
            nc.sync.dma_start(i1A[:], idx1A[:])
            i1B = cpool.tile([P, sched.totB_cols], dt.int16)
            nc.sync.dma_start(i1B[:], idx1B[:])
            i2A = cpool.tile([P, sched.totA_cols], dt.int16)
            nc.sync.dma_start(i2A[:], idx2A[:])
            i2B = cpool.tile([P, sched.totB_cols], dt.int16)
            nc.sync.dma_start(i2B[:], idx2B[:])
            adst_all = cpool.tile([P, NT, 8], dt.bfloat16)
            azdst_all = cpool.tile([P, NT, 1], dt.bfloat16)

            # fused attention weights: wcat = W1 @ [Asrc|Adst]  -> [2,128,16]
            wcat = cpool.tile([P, 2, 16], dt.bfloat16)
            wz = cpool.tile([P, 4, 18], dt.bfloat16)   # [W2 | wcat2] per 128-chunk
            W2s = cpool.tile([P, 4, 16], dt.bfloat16)
            nc.sync.dma_start(W2s[:], W2r[:])
            with tc.tile_pool(name="p0ps", bufs=2, space="PSUM") as p0ps:
                for i in range(2):
                    ps = p0ps.tile([P, 16], dt.float32, tag="wc")
                    for j in range(4):
                        nc.tensor.matmul(
                            ps[:], W1Ts[:, j, i * P:(i + 1) * P], Acats[:, j, :],
                            start=(j == 0), stop=(j == 3),
                        )
                    nc.vector.tensor_copy(wcat[:, i, :], ps[:])
                for cch in range(4):
                    ps2 = p0ps.tile([P, 2], dt.float32, tag="wz")
                    nc.tensor.matmul(
                        ps2[:], W2Ts[:, cch * P:(cch + 1) * P], att2s[:],
                        start=True, stop=True,
                    )
                    nc.vector.tensor_copy(wz[:, cch, 16:18], ps2[:])
                    nc.vector.tensor_copy(wz[:, cch, 0:16], W2s[:, cch, :])

            # ---- P1: replicated node phase --------------------------------
            XB = 4  # node tiles per x load
            with (
                tc.tile_pool(name="p1sb", bufs=3) as p1sb,
                tc.tile_pool(name="p1ps", bufs=2, space="PSUM") as p1ps,
            ):
                for tq in range(0, NODE_TILES, XB):
                    nb = min(XB, NODE_TILES - tq)
                    xt = p1sb.tile([P, 2, nb * P], dt.bfloat16, tag="xt")
                    nc.sync.dma_start(xt[:], xT[:, :, tq * P: tq * P + nb * P])
                    for u in range(nb):
                        ph = p1ps.tile([P, 512], dt.float32, tag="ph")
                        pa = p1ps.tile([P, 16], dt.float32, tag="pa")
                        for c in range(2):
                            lhs = xt[:, c, u * P:(u + 1) * P]
                            nc.tensor.matmul(ph[:], lhs, W1s[:, c, :], start=(c == 0), stop=(c == 1))
                            nc.tensor.matmul(pa[:], lhs, wcat[:, c, :], start=(c == 0), stop=(c == 1))
                        rowt = p1sb.tile([P, ROW1], dt.bfloat16, tag="rowt")
                        nc.scalar.copy(rowt[:, 0:512], ph[:])
                        nc.vector.tensor_copy(rowt[:, 512:528], pa[:])
                        nc.sync.dma_start(
                            table1[(tq + u) * P:(tq + u + 1) * P, :], rowt[:]
                        )

            # ---- P1.5: per-core a_dst table (SBUF, tile-major) ------------
            if phase not in ("p1",):
                rbase = nc.sync.partition_id() * NPC
                nc.sync.dma_start(
                    adst_own[:, :],
                    table1[bass.ds(rbase, NPCP), 520:528],
                )
                nc.sync.dma_start(
                    adst_all[:],
                    adst_own[:].rearrange("(t p) c -> p t c", p=P),
                )

            # ---- edge phase helper ----------------------------------------
            def edge_phase(layer):
                """layer 1: table1 gathers, 8 heads; layer 2: z-table, 1 head."""
                sub = os.environ.get("GAT_L1SUB", "full")
                if layer == 1:
                    g_src_tab_A, g_src_tab_B = table1, table1[sched.split1:, :]
                    g_elem, g_row = ROW1, ROW1
                    iA, iB = i1A, i1B
                    NH = 8
                    adst_t = adst_all
                else:
                    g_src_tab_A, g_src_tab_B = cc_out, cc_out[sched.zsplit:, :]
                    g_elem, g_row = ROWZ, ROWZ
                    iA, iB = i2A, i2B
                    NH = 1
                    adst_t = azdst_all
                MAXTB = sched.maxtb
                with (
                    tc.tile_pool(name=f"ed{layer}", bufs=2) as ep,
                    tc.tile_pool(name=f"sl{layer}", bufs=2) as sp,
                    tc.tile_pool(name=f"eb{layer}", bufs=3) as bp,
                    tc.tile_pool(name=f"ep{layer}", bufs=2, space="PSUM") as pp,
                    tc.tile_pool(name=f"eq{layer}", bufs=1, space="PSUM") as pq,
                ):
                    for g, tl in enumerate(sched.groups):
                        GB = sched.g_tblk[g]
                        GA = sched.g_ablk[g]
                        nA, nB_ = GA * P, (GB - GA) * P
                        hg = ep.tile([P, GB, g_elem], dt.bfloat16, tag="hg")
                        if nA:
                            nc.gpsimd.dma_gather(
                                hg[:, 0:GA, :], g_src_tab_A[:],
                                iA[:, sched.gA_coloff[g]: sched.gA_coloff[g] + nA // 16],
                                nA, nA, g_elem, single_packet=False,
                            )
                        if nB_:
                            nc.gpsimd.dma_gather(
                                hg[:, GA:GB, :], g_src_tab_B,
                                iB[:, sched.gB_coloff[g]: sched.gB_coloff[g] + nB_ // 16],
                                nB_, nB_, g_elem, elem_step=g_row,
                                single_packet=False,
                            )
                        if sub == "gather":
                            continue
                        for ti in tl:
                            ab, bb = int(sched.Ablk[ti]), int(sched.Bblk[ti])
                            tb = ab + bb
                            if tb == 0:
                                continue
                            apos, bpos = sched.tile_apos[ti], sched.tile_bpos[ti]
                            gb = sched.g_base[g]
                            # (in-buffer block, w-slot) pairs for this tile
                            spans = []
                            if ab:
                                spans.append((apos, ab, 0))
                            if bb:
                                spans.append((bpos, bb, ab))
                            # selection matrices + a_dst broadcast via PE:
                            # Sel2 = SelT.T, aDs[e, j, h] = a_dst[dstloc[e], h]
                            selS = sp.tile([P, MAXTB, P], dt.bfloat16, tag="selS")
                            aDs = pq.tile([P, MAXTB, NH], dt.float32, tag="aDs")
                            for (o, n, wo) in spans:
                                for k in range(n):
                                    j = wo + k
                                    gcol = gb + o + k
                                    nc.vector.tensor_scalar(
                                        out=selS[:, j, :], in0=iota_bf[:],
                                        scalar1=dstloc_s[:, gcol:gcol + 1],
                                        scalar2=None, op0=ALU.is_equal,
                                    )
                                    psT = pp.tile([P, P], dt.bfloat16, tag="psT")
                                    nc.tensor.transpose(psT[:], selS[:, j, :], ident[:])
                                    sel2 = bp.tile([P, P], dt.bfloat16, tag="sel2")
                                    nc.scalar.copy(sel2[:], psT[:])
                                    nc.tensor.matmul(
                                        aDs[:, j, :], sel2[:], adst_t[:, ti, :],
                                        start=True, stop=True,
                                    )
                            wt = bp.tile([P, tb * NH], dt.float32, tag="wt")
                            for (o, n, wo) in spans:
                                st = bp.tile([P, n * NH], dt.float32, tag="st")
                                if layer == 1:
                                    a_s = hg[:, o:o + n, 512:520]
                                else:
                                    a_s = hg[:, o:o + n, 17:18]
                                st3 = st[:].rearrange("p (n k) -> p n k", k=NH)
                                nc.vector.tensor_tensor(
                                    out=st3, in0=a_s, in1=aDs[:, wo:wo + n, :], op=ALU.add
                                )
                                lk = bp.tile([P, n * NH], dt.float32, tag="lk")
                                nc.vector.scalar_tensor_tensor(
                                    lk[:], st[:], 0.2, st[:], ALU.mult, ALU.max
                                )
                                nc.scalar.activation(
                                    wt[:, wo * NH:(wo + n) * NH], lk[:], AF.Exp
                                )
                            if layer == 1:
                                wbf = bp.tile([P, tb * NH], dt.bfloat16, tag="wbf")
                                nc.vector.tensor_copy(wbf[:], wt[:])
                            if sub == "score":
                                continue
                            if layer == 1:
                                psum_o = pp.tile([P, 512], dt.float32, tag="psO")
                                psum_d = pp.tile([P, 8], dt.float32, tag="psD")
                            else:
                                psum_o = pp.tile([P, 17], dt.float32, tag="psO")
                                psum_d = None
                            j = 0
                            for (o, n, wo) in spans:
                                for k in range(n):
                                    blk = o + k
                                    jj = wo + k
                                    first, last = (j == 0), (j == tb - 1)
                                    ws = jj * NH
                                    if layer == 1:
                                        M = bp.tile([P, 512], dt.bfloat16, tag="M")
                                        wbc = wbf[:, ws:ws + 8].rearrange(
                                            "p (a b) -> p a b", b=1
                                        ).to_broadcast([P, 8, 64])
                                        nc.vector.tensor_tensor(
                                            out=M[:].rearrange("p (a b) -> p a b", b=64),
                                            in0=hg[:, blk, 0:512].rearrange("p (a b) -> p a b", b=64),
                                            in1=wbc,
                                            op=ALU.mult,
                                        )
                                        nc.tensor.matmul(psum_o[:], selS[:, jj, :], M[:], start=first, stop=last)
                                        nc.tensor.matmul(
                                            psum_d[:], selS[:, jj, :], wbf[:, ws:ws + 8],
                                            start=first, stop=last,
                                        )
                                    else:
                                        # cols 0:16 = z*w, col 16 = w (z-row col 16
                                        # holds 1.0) -> denominator rides along
                                        M = bp.tile([P, 17], dt.bfloat16, tag="M")
                                        nc.vector.tensor_scalar_mul(
                                            M[:], hg[:, blk, 0:17], wt[:, ws:ws + 1]
                                        )
                                        nc.tensor.matmul(psum_o[:], selS[:, jj, :], M[:], start=first, stop=last)
                                    j += 1
                            if sub == "blocks":
                                continue
                            close_tile(layer, ti, psum_o, psum_d, bp, pp, pq)

            # ---- tile close -------------------------------------------------
            def close_tile(layer, ti, psum_o, psum_d, bp, pp, pq):
                if layer == 1:
                    r = bp.tile([P, 8], dt.float32, tag="r")
                    nc.vector.reciprocal(r[:], psum_d[:])
                    o1 = bp.tile([P, 512], dt.bfloat16, tag="o1")
                    for h in range(8):
                        sl = slice(h * 64, (h + 1) * 64)
                        if h % 2 == 0:
                            nc.scalar.activation(
                                o1[:, sl], psum_o[:, sl], AF.Copy, scale=r[:, h:h + 1]
                            )
                        else:
                            nc.vector.tensor_scalar_mul(o1[:, sl], psum_o[:, sl], r[:, h:h + 1])
                    # elu: h2 = max(o1,0) + exp(min(o1,0)) - 1
                    u = bp.tile([P, 512], dt.bfloat16, tag="u")
                    nc.vector.tensor_scalar_min(u[:], o1[:], 0.0)
                    e1 = bp.tile([P, 512], dt.bfloat16, tag="e1")
                    nc.scalar.activation(e1[:], u[:], AF.Exp)
                    rv = bp.tile([P, 512], dt.bfloat16, tag="rv")
                    nc.vector.tensor_scalar_max(rv[:], o1[:], 0.0)
                    h2 = bp.tile([P, 512], dt.bfloat16, tag="h2")
                    nc.vector.scalar_tensor_tensor(
                        h2[:], e1[:], -1.0, rv[:], ALU.add, ALU.add
                    )
                    # transpose h2 -> z matmuls
                    pz = pq.tile([P, 18], dt.float32, tag="psZ")
                    for c in range(4):
                        ptr = pp.tile([P, P], dt.bfloat16, tag="psT")
                        nc.tensor.transpose(ptr[:], h2[:, c * P:(c + 1) * P], ident[:])
                        h2T = bp.tile([P, P], dt.bfloat16, tag="h2T")
                        nc.scalar.copy(h2T[:], ptr[:])
                        nc.tensor.matmul(pz[:], h2T[:], wz[:, c, :], start=(c == 0), stop=(c == 3))
                    # z-row layout: [z 16 | 1.0 | a_src2 | a_dst2 | junk]
                    zrow = bp.tile([P, ROWZ], dt.bfloat16, tag="zrow")
                    nc.vector.tensor_copy(zrow[:, 0:16], pz[:, 0:16])
                    nc.vector.memset(zrow[:, 16:17], 1.0)
                    nc.vector.tensor_copy(zrow[:, 17:19], pz[:, 16:18])
                    nc.sync.dma_start(cc_in[ti * P:(ti + 1) * P, :], zrow[:])
                else:
                    r2 = bp.tile([P, 1], dt.float32, tag="r2")
                    nc.vector.reciprocal(r2[:], psum_o[:, 16:17])
                    o2 = bp.tile([P, 16], dt.float32, tag="o2")
                    nc.vector.tensor_scalar_mul(o2[:], psum_o[:, 0:16], r2[:, 0:1])
                    mx = bp.tile([P, 1], dt.float32, tag="mx")
                    nc.vector.tensor_reduce(mx[:], o2[:], axis=mybir.AxisListType.X, op=ALU.max)
                    o2m = bp.tile([P, 16], dt.float32, tag="o2m")
                    nc.vector.tensor_scalar_sub(o2m[:], o2[:], mx[:, 0:1])
                    ex = bp.tile([P, 16], dt.float32, tag="ex")
                    ssum = bp.tile([P, 1], dt.float32, tag="ssum")
                    nc.scalar.activation(ex[:], o2m[:], AF.Exp, accum_out=ssum[:])
                    lse = bp.tile([P, 1], dt.float32, tag="lse")
                    nc.scalar.activation(lse[:], ssum[:], AF.Ln)
                    res = bp.tile([P, 16], dt.float32, tag="res")
                    nc.vector.tensor_scalar_sub(res[:], o2m[:], lse[:, 0:1])
                    nc.sync.dma_start(out_shard[ti * P:(ti + 1) * P, :], res[:])

            if phase not in ("p1", "p15"):
                edge_phase(1)

            if phase in ("cc", "full"):
                # ---- z-table exchange -------------------------------------
                if n_cores == 1:
                    nc.sync.dma_start(cc_out[:, :], cc_in[:, :])
                else:
                    nc.gpsimd.collective_compute(
                        "AllGather", ALU.bypass,
                        ins=[cc_in[:]], outs=[cc_out[:]],
                        replica_groups=[list(range(n_cores))],
                    )

            if phase == "full":
                nc.sync.dma_start(
                    azdst_all[:],
                    cc_in[:, 18:19].rearrange("(t p) c -> p t c", p=P),
                )
                edge_phase(2)

    nc.compile()
    return nc


# ----------------------------------------------------------------------------
# host entry
# ----------------------------------------------------------------------------

def _blockdiag(att, heads, hid):
    """[heads, hid] -> [heads*hid, heads] block diagonal."""
    out = np.zeros((heads * hid, max(heads, 1)), np.float32)
    for h in range(heads):
        out[h * hid:(h + 1) * hid, h] = att[h]
    return out


def prepare_inputs(inputs, sched: Schedule):
    x = np.asarray(inputs["x"], np.float32)
    ei = np.asarray(inputs["edge_index"])
    W1 = np.asarray(inputs["W1"], np.float32)
    as1 = np.asarray(inputs["att_src1"], np.float32)
    ad1 = np.asarray(inputs["att_dst1"], np.float32)
    W2 = np.asarray(inputs["W2"], np.float32)
    as2 = np.asarray(inputs["att_src2"], np.float32)
    ad2 = np.asarray(inputs["att_dst2"], np.float32)

    N, IN = x.shape
    TR = sched.table_rows
    xp = np.zeros((TR, IN), np.float32)
    xp[:N] = x
    xTb = np.ascontiguousarray(
        xp.T.reshape(2, P, TR).transpose(1, 0, 2)).astype(BF)
    W1b = np.ascontiguousarray(W1.reshape(2, P, 512).transpose(1, 0, 2)).astype(BF)
    W1Tb = np.ascontiguousarray(
        W1.T.reshape(4, P, 256).transpose(1, 0, 2)).astype(BF)
    AcatB = np.ascontiguousarray(
        np.concatenate([_blockdiag(as1, 8, 64), _blockdiag(ad1, 8, 64)], axis=1)
        .reshape(4, P, 16).transpose(1, 0, 2)
    ).astype(BF)
    W2b = np.ascontiguousarray(W2.reshape(4, P, 16).transpose(1, 0, 2)).astype(BF)
    W2Tb = np.ascontiguousarray(W2.T).astype(BF)          # [16, 512]
    att2b = np.concatenate([as2.T, ad2.T], axis=1).astype(BF)  # [16, 2]

    shared = dict(xT=xTb, W1r=W1b, W1Tr=W1Tb, Acat=AcatB, W2r=W2b, W2Tr=W2Tb, att2=att2b)
    maps = []
    for c in range(sched.n_cores):
        pc = sched.per_core[c]
        m = dict(shared)
        m.update(
            idx1A=pc["idx1A"], idx1B=pc["idx1B"], idx2A=pc["idx2A"],
            idx2B=pc["idx2B"], dstloc=pc["dstloc"],
        )
        maps.append(m)
    return maps


_LAST_RESULT = {}


def kernel(**inputs):
    from concourse.bass_utils import run_bass_kernel_spmd

    x = np.asarray(inputs["x"], np.float32)
    ei = np.asarray(inputs["edge_index"], np.int64)
    N = x.shape[0]
    n_cores = 8
    loops = np.arange(N, dtype=np.int64)
    src = np.concatenate([ei[0], loops])
    dst = np.concatenate([ei[1], loops])

    sched = Schedule(src, dst, N, n_cores)
    phase = os.environ.get("GAT_PHASE", "full")
    nc = build_program(sched, n_cores, phase=phase)
    in_maps = prepare_inputs(inputs, sched)

    trace = bool(int(os.environ.get("GAT_TRACE", "0")))
    res = run_bass_kernel_spmd(
        nc, in_maps, core_ids=list(range(n_cores)), trace=trace,
    )
    _LAST_RESULT["res"] = res

    out = np.zeros((N, 16), np.float32)
    for c in range(n_cores):
        sh = res.results[c]["out_shard"]
        n0 = c * sched.npc
        out[n0:n0 + sched.npc] = sh[: sched.npc]
    return out



# revision 21
# speedup vs baseline: 1.3076x; 1.2660x over previous
"""2-layer GAT (GATConv x2 + log_softmax) on 8 Trainium2 NeuronCores.

Strategy (dst-sharded message passing):
  - Nodes are sharded contiguously across 8 cores (6250 each); every edge is
    owned by the core owning its dst node.  Edges are grouped by dst tile
    (128 dst nodes), split into A/B halves by src id (so gather indices fit
    int16), padded to 128-edge blocks with a cross-core-uniform schedule so
    all 8 cores run one SPMD program.
  - Layer-1 node phase is replicated: every core computes h = x@W1 (bf16,
    fp32 accum) for ALL nodes and writes a gather table
    [h(512) | a_src(8) | a_dst(8) | pad] bf16 per node.
  - Edge phase per 128-edge block: dma_gather rows by src, build a 0/1
    selection matrix SelT[e,d] = (dst_local[e] == d) on DVE, per-head
    weight multiply, then PE matmul SelT.T @ M accumulates the segment sum
    (and the softmax denominator) in PSUM per dst tile.
  - Scores: exp(leaky_relu(a_src[src] + a_dst[dst])) with a_dst gathered
    from a per-core table; softmax normalization is applied per dst tile
    after aggregation (alpha = w/denom pulled out of the edge sum).
  - Layer 2 (1 head, 16 ch) reuses the same block structure; the small
    z-table is exchanged with an AllGather collective.
"""
import os
import math
import numpy as np
import ml_dtypes

import concourse.bass as bass
import concourse.mybir as mybir
import concourse.tile as tile
import concourse.bacc as bacc
from concourse.masks import make_identity
from concourse.library_config import mlp

BF = ml_dtypes.bfloat16
dt = mybir.dt
AF = mybir.ActivationFunctionType
ALU = mybir.AluOpType

P = 128
ROW1 = 640     # table1 cols (bf16): [h 512 | a_src 8 | a_dst 8 | pad]
ROWZ = 128     # z-table cols (bf16): [z 16 | a_src2 1 | a_dst2 1 | pad]
BLKCAP = 40    # max blocks per gather group


# ----------------------------------------------------------------------------
# host-side schedule construction
# ----------------------------------------------------------------------------

def _wrap_idx(vals, slots):
    """Pad `vals` with 0 to `slots`, wrap into [128, slots/16] int16 layout."""
    v = np.zeros(slots, np.int64)
    v[: len(vals)] = vals
    a = v.reshape(-1, 16).T  # [16, slots/16]
    return np.tile(a, (8, 1)).astype(np.int16)


class Schedule:
    """Cross-core-uniform block schedule + per-core index arrays."""

    def __init__(self, src, dst, n_nodes, n_cores, force_split=None):
        self.n_nodes = n_nodes
        self.n_cores = n_cores
        self.npc = n_nodes // n_cores                 # real nodes per core
        self.nt = (self.npc + P - 1) // P             # dst tiles per core
        self.npcp = self.nt * P                       # padded nodes per core
        self.ntot_p = ((n_nodes + P - 1) // P) * P if n_cores == 1 else None
        # padded global table rows (node-id indexed)
        self.table_rows = ((n_nodes + P - 1) // P) * P
        self.table_rows = max(self.table_rows, self.npcp * n_cores)
        self.zrows = self.npcp * n_cores              # z-table rows (zid indexed)

        # split for int16 gathers: src <= SPLIT1-1 -> table A half;
        # zid(src) <= 32767 must also hold.
        if force_split is not None:
            self.split1 = force_split
        elif self.table_rows <= 32768 and self.zrows <= 32768:
            self.split1 = self.table_rows  # no B half
        else:
            # largest s with s-1 <= 32767 and zid(s-1) <= 32767
            s = min(32768, self.n_nodes)
            while s > 0:
                n = s - 1
                zid = (n // self.npc) * self.npcp + (n % self.npc)
                if zid <= 32767:
                    break
                s -= 1
            self.split1 = s
        self.zsplit = ((self.split1 - 1) // self.npc) * self.npcp + (
            (self.split1 - 1) % self.npc
        ) + 1 if self.split1 < self.table_rows else self.zrows

        core = dst // self.npc
        loc = dst - core * self.npc
        t = loc // P
        dloc = loc % P
        isB = src >= self.split1

        nc_, nt_ = n_cores, self.nt
        # counts[core, tile, {A,B}]
        key = (core * nt_ + t) * 2 + isB
        cnt = np.bincount(key, minlength=nc_ * nt_ * 2).reshape(nc_, nt_, 2)
        mx = cnt.max(axis=0)                            # [nt, 2]
        self.Ablk = np.ceil(mx[:, 0] / P).astype(int)
        self.Bblk = np.ceil(mx[:, 1] / P).astype(int)
        self.TBlk = self.Ablk + self.Bblk

        # groups: consecutive tiles, sum(TBlk) <= BLKCAP
        self.groups = []
        cur, acc = [], 0
        for ti in range(nt_):
            tb = int(self.TBlk[ti])
            if cur and acc + tb > BLKCAP:
                self.groups.append(cur)
                cur, acc = [], 0
            cur.append(ti)
            acc += tb
        if cur:
            self.groups.append(cur)

        # canonical block order & per-tile positions within group buffers
        # group buffer layout: [A-blocks of each tile in order, then B-blocks]
        self.g_ablk = []   # per group: total A blocks
        self.g_tblk = []   # per group: total blocks
        self.tile_apos = {}  # tile -> in-group A block offset
        self.tile_bpos = {}  # tile -> in-group block offset of its B blocks
        self.g_base = []     # per group: global block offset
        nblocks = 0
        for g, tl in enumerate(self.groups):
            ga = int(sum(self.Ablk[ti] for ti in tl))
            gt = int(sum(self.TBlk[ti] for ti in tl))
            self.g_ablk.append(ga)
            self.g_tblk.append(gt)
            ao = 0
            bo = ga
            for ti in tl:
                self.tile_apos[ti] = ao
                self.tile_bpos[ti] = bo
                ao += int(self.Ablk[ti])
                bo += int(self.Bblk[ti])
            self.g_base.append(nblocks)
            nblocks += gt
        self.nblocks = nblocks

        self.maxtb = int(self.TBlk.max())

        # per-core arrays
        # order edges by (core, tile, isB) stably
        order = np.lexsort((isB, t, core))
        self.per_core = []
        for c in range(nc_):
            m0 = order[core[order] == c]
            idx1A_cols, idx1B_cols, idx2A_cols, idx2B_cols = [], [], [], []
            dstloc = np.full((P, nblocks), 999.0, np.float32)
            for g, tl in enumerate(self.groups):
                a_src_l, b_src_l = [], []
                dl_A, dl_B = [], []
                for ti in tl:
                    e = m0[t[m0] == ti]
                    eA = e[~isB[e]]
                    eB = e[isB[e]]
                    nA = int(self.Ablk[ti]) * P
                    nB = int(self.Bblk[ti]) * P
                    sA = np.zeros(nA, np.int64)
                    sA[: len(eA)] = src[eA]
                    sB = np.zeros(nB, np.int64)
                    sB[: len(eB)] = src[eB] - self.split1
                    lA = np.full(nA, 999.0, np.float32)
                    lA[: len(eA)] = dloc[eA]
                    lB = np.full(nB, 999.0, np.float32)
                    lB[: len(eB)] = dloc[eB]
                    a_src_l.append(sA)
                    b_src_l.append(sB)
                    dl_A.append(lA)
                    dl_B.append(lB)
                gsA = np.concatenate(a_src_l) if a_src_l else np.zeros(0, np.int64)
                gsB = np.concatenate(b_src_l) if b_src_l else np.zeros(0, np.int64)
                gdl = np.concatenate(dl_A + dl_B) if (dl_A or dl_B) else np.zeros(0, np.float32)
                # L2 indices: zid mapping of global src
                def zid_of(v):
                    vv = np.asarray(v, np.int64)
                    return (vv // self.npc) * self.npcp + (vv % self.npc)
                g2A = zid_of(gsA)                       # gsA holds global src (pads=0)
                g2B = zid_of(gsB + self.split1) - self.zsplit
                idx1A_cols.append(_wrap_idx(gsA, len(gsA)))
                idx1B_cols.append(_wrap_idx(gsB, len(gsB)))
                idx2A_cols.append(_wrap_idx(g2A, len(g2A)))
                idx2B_cols.append(_wrap_idx(g2B, len(g2B)))
                gb = self.g_base[g]
                dstloc[:, gb : gb + self.g_tblk[g]] = gdl.reshape(-1, P).T
            cat = lambda ls: (
                np.concatenate(ls, axis=1) if ls and sum(x.shape[1] for x in ls) else np.zeros((P, 1), np.int16)
            )
            # partition-replicated transposed dstloc (pad -> 512, exact in bf16)
            dl = dstloc.T.copy()                       # [nblocks, P(edge)]
            dl[dl == 999.0] = 512.0
            dstlocT = np.broadcast_to(
                dl[None, :, :], (P, nblocks, P)
            ).astype(BF).copy()
            self.per_core.append(
                dict(
                    idx1A=cat(idx1A_cols), idx1B=cat(idx1B_cols),
                    idx2A=cat(idx2A_cols), idx2B=cat(idx2B_cols),
                    dstloc=dstloc, dstlocT=dstlocT,
                )
            )
        # column offsets per group in the concatenated idx arrays
        self.gA_coloff, self.gB_coloff = [], []
        a = b = 0
        for g in range(len(self.groups)):
            self.gA_coloff.append(a)
            self.gB_coloff.append(b)
            a += (self.g_ablk[g] * P) // 16
            b += ((self.g_tblk[g] - self.g_ablk[g]) * P) // 16
        self.totA_cols = max(a, 1)
        self.totB_cols = max(b, 1)


# ----------------------------------------------------------------------------
# device program
# ----------------------------------------------------------------------------

def build_program(sched: Schedule, n_cores: int, phase: str = 'full'):
    """Build the SPMD Bass/Tile program for the given schedule."""
    nc = bacc.Bacc(None, target_bir_lowering=False, debug=True, num_devices=n_cores)

    TR = sched.table_rows
    ZR = sched.zrows
    NT = sched.nt
    NPC, NPCP = sched.npc, sched.npcp
    NODE_TILES = TR // P

    # ---- inputs -------------------------------------------------------------
    xT = nc.dram_tensor("xT", [P, 2, TR], dt.bfloat16, kind="ExternalInput")
    W1r = nc.dram_tensor("W1r", [P, 2, 512], dt.bfloat16, kind="ExternalInput")
    W1Tr = nc.dram_tensor("W1Tr", [P, 4, 256], dt.bfloat16, kind="ExternalInput")
    Acat = nc.dram_tensor("Acat", [P, 4, 16], dt.bfloat16, kind="ExternalInput")
    W2r = nc.dram_tensor("W2r", [P, 4, 16], dt.bfloat16, kind="ExternalInput")
    W2Tr = nc.dram_tensor("W2Tr", [16, 512], dt.bfloat16, kind="ExternalInput")
    att2 = nc.dram_tensor("att2", [16, 2], dt.bfloat16, kind="ExternalInput")
    idx1A = nc.dram_tensor("idx1A", [P, sched.totA_cols], dt.int16, kind="ExternalInput")
    idx1B = nc.dram_tensor("idx1B", [P, sched.totB_cols], dt.int16, kind="ExternalInput")
    idx2A = nc.dram_tensor("idx2A", [P, sched.totA_cols], dt.int16, kind="ExternalInput")
    idx2B = nc.dram_tensor("idx2B", [P, sched.totB_cols], dt.int16, kind="ExternalInput")
    dstlocr = nc.dram_tensor("dstloc", [P, sched.nblocks], dt.float32, kind="ExternalInput")
    dstlocTr = nc.dram_tensor("dstlocT", [P, sched.nblocks, P], dt.bfloat16, kind="ExternalInput")
    out_shard = nc.dram_tensor("out_shard", [NPCP, 16], dt.float32, kind="ExternalOutput")

    with tile.TileContext(nc) as tc:
        nc.gpsimd.load_library(mlp)
        with (
            tc.tile_pool(name="dram", bufs=1, space="DRAM") as dram,
            tc.tile_pool(name="const", bufs=1) as cpool,
        ):
            table1 = dram.tile([TR, ROW1], dt.bfloat16)
            adst_own = dram.tile([NPCP, 8], dt.bfloat16)
            cc_in = dram.tile([NPCP, ROWZ], dt.bfloat16)
            cc_out = dram.tile([ZR, ROWZ], dt.bfloat16,
                               addr_space=("Shared" if n_cores > 1 else "Local"))

            # ---- constants -------------------------------------------------
            iota_i = cpool.tile([P, P], dt.int32)
            nc.gpsimd.iota(iota_i[:], pattern=[[1, P]], base=0, channel_multiplier=0)
            iota_bf = cpool.tile([P, P], dt.bfloat16)
            nc.vector.tensor_copy(iota_bf[:], iota_i[:])
            iota_ci = cpool.tile([P, 1], dt.int32)
            nc.gpsimd.iota(iota_ci[:], pattern=[[0, 1]], base=0, channel_multiplier=1)
            iota_cf = cpool.tile([P, 1], dt.float32)
            nc.vector.tensor_copy(iota_cf[:], iota_ci[:])
            ident = cpool.tile([P, P], dt.bfloat16)
            make_identity(nc, ident[:])

            W1s = cpool.tile([P, 2, 512], dt.bfloat16)
            nc.sync.dma_start(W1s[:], W1r[:])
            W1Ts = cpool.tile([P, 4, 256], dt.bfloat16)
            nc.sync.dma_start(W1Ts[:], W1Tr[:])
            Acats = cpool.tile([P, 4, 16], dt.bfloat16)
            nc.sync.dma_start(Acats[:], Acat[:])
            W2Ts = cpool.tile([16, 512], dt.bfloat16)
            nc.sync.dma_start(W2Ts[:], W2Tr[:])
            att2s = cpool.tile([16, 2], dt.bfloat16)
            nc.sync.dma_start(att2s[:], att2[:])
            dstloc_s = cpool.tile([P, sched.nblocks], dt.float32)
            nc.sync.dma_start(dstloc_s[:], dstlocr[:])
            i1A = cpool.tile([P, sched.totA_cols], dt.int16)
            nc.sync.dma_start(i1A[:], idx1A[:])
            i1B = cpool.tile([P, sched.totB_cols], dt.int16)
            nc.sync.dma_start(i1B[:], idx1B[:])
            i2A = cpool.tile([P, sched.totA_cols], dt.int16)
            nc.sync.dma_start(i2A[:], idx2A[:])
            i2B = cpool.tile([P, sched.totB_cols], dt.int16)
            nc.sync.dma_start(i2B[:], idx2B[:])
            adst_all = cpool.tile([P, NT, 8], dt.bfloat16)
            azdst_all = cpool.tile([P, NT, 1], dt.bfloat16)

            # fused attention weights: wcat = W1 @ [Asrc|Adst]  -> [2,128,16]
            wcat = cpool.tile([P, 2, 16], dt.bfloat16)
            wz = cpool.tile([P, 4, 18], dt.bfloat16)   # [W2 | wcat2] per 128-chunk
            W2s = cpool.tile([P, 4, 16], dt.bfloat16)
            nc.sync.dma_start(W2s[:], W2r[:])
            with tc.tile_pool(name="p0ps", bufs=2, space="PSUM") as p0ps:
                for i in range(2):
                    ps = p0ps.tile([P, 16], dt.float32, tag="wc")
                    for j in range(4):
                        nc.tensor.matmul(
                            ps[:], W1Ts[:, j, i * P:(i + 1) * P], Acats[:, j, :],
                            start=(j == 0), stop=(j == 3),
                        )
                    nc.vector.tensor_copy(wcat[:, i, :], ps[:])
                for cch in range(4):
                    ps2 = p0ps.tile([P, 2], dt.float32, tag="wz")
                    nc.tensor.matmul(
                        ps2[:], W2Ts[:, cch * P:(cch + 1) * P], att2s[:],
                        start=True, stop=True,
                    )
                    nc.vector.tensor_copy(wz[:, cch, 16:18], ps2[:])
                    nc.vector.tensor_copy(wz[:, cch, 0:16], W2s[:, cch, :])

            # ---- P1: replicated node phase --------------------------------
            XB = 4  # node tiles per x load
            with (
                tc.tile_pool(name="p1sb", bufs=3) as p1sb,
                tc.tile_pool(name="p1ps", bufs=2, space="PSUM") as p1ps,
            ):
                for tq in range(0, NODE_TILES, XB):
                    nb = min(XB, NODE_TILES - tq)
                    xt = p1sb.tile([P, 2, nb * P], dt.bfloat16, tag="xt")
                    nc.sync.dma_start(xt[:], xT[:, :, tq * P: tq * P + nb * P])
                    for u in range(nb):
                        ph = p1ps.tile([P, 512], dt.float32, tag="ph")
                        pa = p1ps.tile([P, 16], dt.float32, tag="pa")
                        for c in range(2):
                            lhs = xt[:, c, u * P:(u + 1) * P]
                            nc.tensor.matmul(ph[:], lhs, W1s[:, c, :], start=(c == 0), stop=(c == 1))
                            nc.tensor.matmul(pa[:], lhs, wcat[:, c, :], start=(c == 0), stop=(c == 1))
                        rowt = p1sb.tile([P, ROW1], dt.bfloat16, tag="rowt")
                        nc.scalar.copy(rowt[:, 0:512], ph[:])
                        nc.vector.tensor_copy(rowt[:, 512:528], pa[:])
                        nc.sync.dma_start(
                            table1[(tq + u) * P:(tq + u + 1) * P, :], rowt[:]
                        )

            # ---- P1.5: per-core a_dst table (SBUF, tile-major) ------------
            if phase not in ("p1",):
                rbase = nc.sync.partition_id() * NPC
                nc.sync.dma_start(
                    adst_own[:, :],
                    table1[bass.ds(rbase, NPCP), 520:528],
                )
                nc.sync.dma_start(
                    adst_all[:],
                    adst_own[:].rearrange("(t p) c -> p t c", p=P),
                )

            # ---- edge phase helper ----------------------------------------
            def edge_phase(layer):
                """layer 1: table1 gathers, 8 heads; layer 2: z-table, 1 head."""
                sub = os.environ.get("GAT_L1SUB", "full")
                if layer == 1:
                    g_src_tab_A, g_src_tab_B = table1, table1[sched.split1:, :]
                    g_elem, g_row = ROW1, ROW1
                    iA, iB = i1A, i1B
                    NH = 8
                    adst_t = adst_all
                else:
                    g_src_tab_A, g_src_tab_B = cc_out, cc_out[sched.zsplit:, :]
                    g_elem, g_row = ROWZ, ROWZ
                    iA, iB = i2A, i2B
                    NH = 1
                    adst_t = azdst_all
                MAXTB = sched.maxtb
                with (
                    tc.tile_pool(name=f"ed{layer}", bufs=2) as ep,
                    tc.tile_pool(name=f"sl{layer}", bufs=2) as sp,
                    tc.tile_pool(name=f"eb{layer}", bufs=3) as bp,
                    tc.tile_pool(name=f"ep{layer}", bufs=2, space="PSUM") as pp,
                    tc.tile_pool(name=f"eo{layer}", bufs=1, space="PSUM") as po,
                    tc.tile_pool(name=f"eq{layer}", bufs=2, space="PSUM") as pq,
                ):
                    for g, tl in enumerate(sched.groups):
                        GB = sched.g_tblk[g]
                        GA = sched.g_ablk[g]
                        nA, nB_ = GA * P, (GB - GA) * P
                        gb = sched.g_base[g]
                        hg = ep.tile([P, GB, g_elem], dt.bfloat16, tag="hg")
                        if nA:
                            nc.gpsimd.dma_gather(
                                hg[:, 0:GA, :], g_src_tab_A[:],
                                iA[:, sched.gA_coloff[g]: sched.gA_coloff[g] + nA // 16],
                                nA, nA, g_elem, single_packet=False,
                            )
                        if nB_:
                            nc.gpsimd.dma_gather(
                                hg[:, GA:GB, :], g_src_tab_B,
                                iB[:, sched.gB_coloff[g]: sched.gB_coloff[g] + nB_ // 16],
                                nB_, nB_, g_elem, elem_step=g_row,
                                single_packet=False,
                            )
                        # transposed dstloc rows for this group (HWDGE stream)
                        dlT = ep.tile([P, GB, P], dt.bfloat16, tag="dlT")
                        nc.sync.dma_start(dlT[:], dstlocTr[:, gb:gb + GB, :])
                        if sub == "gather":
                            continue
                        for ti in tl:
                            ab, bb = int(sched.Ablk[ti]), int(sched.Bblk[ti])
                            tb = ab + bb
                            if tb == 0:
                                continue
                            apos, bpos = sched.tile_apos[ti], sched.tile_bpos[ti]
                            # (in-buffer block, w-slot) pairs for this tile
                            spans = []
                            if ab:
                                spans.append((apos, ab, 0))
                            if bb:
                                spans.append((bpos, bb, ab))
                            # selection matrices (both orientations, DVE only) +
                            # a_dst broadcast: aDs[e, j, h] = a_dst[dstloc[e], h]
                            selS = sp.tile([P, MAXTB, P], dt.bfloat16, tag="selS")
                            aDs = pq.tile([P, MAXTB, NH], dt.float32, tag="aDs")
                            for (o, n, wo) in spans:
                                for k in range(n):
                                    j = wo + k
                                    gcol = gb + o + k
                                    nc.vector.tensor_scalar(
                                        out=selS[:, j, :], in0=iota_bf[:],
                                        scalar1=dstloc_s[:, gcol:gcol + 1],
                                        scalar2=None, op0=ALU.is_equal,
                                    )
                                    sel2 = bp.tile([P, P], dt.bfloat16, tag="sel2")
                                    nc.vector.tensor_scalar(
                                        out=sel2[:], in0=dlT[:, o + k, :],
                                        scalar1=iota_cf[:, 0:1],
                                        scalar2=None, op0=ALU.is_equal,
                                    )
                                    nc.tensor.matmul(
                                        aDs[:, j, :], sel2[:], adst_t[:, ti, :],
                                        start=True, stop=True,
                                    )
                            wt = bp.tile([P, tb * NH], dt.float32, tag="wt")
                            for (o, n, wo) in spans:
                                st = bp.tile([P, n * NH], dt.float32, tag="st")
                                if layer == 1:
                                    a_s = hg[:, o:o + n, 512:520]
                                else:
                                    a_s = hg[:, o:o + n, 17:18]
                                st3 = st[:].rearrange("p (n k) -> p n k", k=NH)
                                nc.vector.tensor_tensor(
                                    out=st3, in0=a_s, in1=aDs[:, wo:wo + n, :], op=ALU.add
                                )
                                lk = bp.tile([P, n * NH], dt.float32, tag="lk")
                                nc.vector.scalar_tensor_tensor(
                                    lk[:], st[:], 0.2, st[:], ALU.mult, ALU.max
                                )
                                nc.scalar.activation(
                                    wt[:, wo * NH:(wo + n) * NH], lk[:], AF.Exp
                                )
                            if layer == 1:
                                wbf = bp.tile([P, tb * NH], dt.bfloat16, tag="wbf")
                                nc.vector.tensor_copy(wbf[:], wt[:])
                            if sub == "score":
                                continue
                            if layer == 1:
                                psum_o = pp.tile([P, 512], dt.float32, tag="psO")
                                psum_d = po.tile([P, 8], dt.float32, tag="psD")
                            else:
                                psum_o = pp.tile([P, 17], dt.float32, tag="psO")
                                psum_d = None
                            j = 0
                            for (o, n, wo) in spans:
                                for k in range(n):
                                    blk = o + k
                                    jj = wo + k
                                    first, last = (j == 0), (j == tb - 1)
                                    ws = jj * NH
                                    if layer == 1:
                                        M = bp.tile([P, 512], dt.bfloat16, tag="M")
                                        wbc = wbf[:, ws:ws + 8].rearrange(
                                            "p (a b) -> p a b", b=1
                                        ).to_broadcast([P, 8, 64])
                                        nc.vector.tensor_tensor(
                                            out=M[:].rearrange("p (a b) -> p a b", b=64),
                                            in0=hg[:, blk, 0:512].rearrange("p (a b) -> p a b", b=64),
                                            in1=wbc,
                                            op=ALU.mult,
                                        )
                                        nc.tensor.matmul(psum_o[:], selS[:, jj, :], M[:], start=first, stop=last)
                                        nc.tensor.matmul(
                                            psum_d[:], selS[:, jj, :], wbf[:, ws:ws + 8],
                                            start=first, stop=last,
                                        )
                                    else:
                                        # cols 0:16 = z*w, col 16 = w (z-row col 16
                                        # holds 1.0) -> denominator rides along
                                        M = bp.tile([P, 17], dt.bfloat16, tag="M")
                                        nc.vector.tensor_scalar_mul(
                                            M[:], hg[:, blk, 0:17], wt[:, ws:ws + 1]
                                        )
                                        nc.tensor.matmul(psum_o[:], selS[:, jj, :], M[:], start=first, stop=last)
                                    j += 1
                            if sub == "blocks":
                                continue
                            close_tile(layer, ti, psum_o, psum_d, bp, pp, po)

            # ---- tile close -------------------------------------------------
            def close_tile(layer, ti, psum_o, psum_d, bp, pp, pq):
                if layer == 1:
                    r = bp.tile([P, 8], dt.float32, tag="r")
                    nc.vector.reciprocal(r[:], psum_d[:])
                    o1 = bp.tile([P, 512], dt.bfloat16, tag="o1")
                    o13 = o1[:].rearrange("p (a b) -> p a b", b=64)
                    rbc = r[:].rearrange("p (a b) -> p a b", b=1).to_broadcast([P, 8, 64])
                    nc.vector.tensor_tensor(
                        out=o13,
                        in0=psum_o[:].rearrange("p (a b) -> p a b", b=64),
                        in1=rbc, op=ALU.mult,
                    )
                    # elu: h2 = max(o1,0) + exp(min(o1,0)) - 1
                    u = bp.tile([P, 512], dt.bfloat16, tag="u")
                    nc.vector.tensor_scalar_min(u[:], o1[:], 0.0)
                    e1 = bp.tile([P, 512], dt.bfloat16, tag="e1")
                    nc.scalar.activation(e1[:], u[:], AF.Exp)
                    rv = bp.tile([P, 512], dt.bfloat16, tag="rv")
                    nc.vector.tensor_scalar_max(rv[:], o1[:], 0.0)
                    h2 = bp.tile([P, 512], dt.bfloat16, tag="h2")
                    nc.vector.scalar_tensor_tensor(
                        h2[:], e1[:], -1.0, rv[:], ALU.add, ALU.add
                    )
                    # transpose h2 -> z matmuls
                    pz = pq.tile([P, 18], dt.float32, tag="psZ")
                    for c in range(4):
                        ptr = pp.tile([P, P], dt.bfloat16, tag="psT")
                        nc.tensor.transpose(ptr[:], h2[:, c * P:(c + 1) * P], ident[:])
                        h2T = bp.tile([P, P], dt.bfloat16, tag="h2T")
                        nc.scalar.copy(h2T[:], ptr[:])
                        nc.tensor.matmul(pz[:], h2T[:], wz[:, c, :], start=(c == 0), stop=(c == 3))
                    # z-row layout: [z 16 | 1.0 | a_src2 | a_dst2 | junk]
                    zrow = bp.tile([P, ROWZ], dt.bfloat16, tag="zrow")
                    nc.vector.tensor_copy(zrow[:, 0:16], pz[:, 0:16])
                    nc.vector.memset(zrow[:, 16:17], 1.0)
                    nc.vector.tensor_copy(zrow[:, 17:19], pz[:, 16:18])
                    nc.sync.dma_start(cc_in[ti * P:(ti + 1) * P, :], zrow[:])
                else:
                    r2 = bp.tile([P, 1], dt.float32, tag="r2")
                    nc.vector.reciprocal(r2[:], psum_o[:, 16:17])
                    o2 = bp.tile([P, 16], dt.float32, tag="o2")
                    nc.vector.tensor_scalar_mul(o2[:], psum_o[:, 0:16], r2[:, 0:1])
                    mx = bp.tile([P, 1], dt.float32, tag="mx")
                    nc.vector.tensor_reduce(mx[:], o2[:], axis=mybir.AxisListType.X, op=ALU.max)
                    o2m = bp.tile([P, 16], dt.float32, tag="o2m")
                    nc.vector.tensor_scalar_sub(o2m[:], o2[:], mx[:, 0:1])
                    ex = bp.tile([P, 16], dt.float32, tag="ex")
                    ssum = bp.tile([P, 1], dt.float32, tag="ssum")
                    nc.scalar.activation(ex[:], o2m[:], AF.Exp, accum_out=ssum[:])
                    lse = bp.tile([P, 1], dt.float32, tag="lse")
                    nc.scalar.activation(lse[:], ssum[:], AF.Ln)
                    res = bp.tile([P, 16], dt.float32, tag="res")
                    nc.vector.tensor_scalar_sub(res[:], o2m[:], lse[:, 0:1])
                    nc.sync.dma_start(out_shard[ti * P:(ti + 1) * P, :], res[:])

            if phase not in ("p1", "p15"):
                edge_phase(1)

            if phase in ("cc", "full"):
                # ---- z-table exchange -------------------------------------
                if n_cores == 1:
                    nc.sync.dma_start(cc_out[:, :], cc_in[:, :])
                else:
                    nc.gpsimd.collective_compute(
                        "AllGather", ALU.bypass,
                        ins=[cc_in[:]], outs=[cc_out[:]],
                        replica_groups=[list(range(n_cores))],
                    )

            if phase == "full":
                nc.sync.dma_start(
                    azdst_all[:],
                    cc_in[:, 18:19].rearrange("(t p) c -> p t c", p=P),
                )
                edge_phase(2)

    nc.compile()
    return nc


# ----------------------------------------------------------------------------
# host entry
# ----------------------------------------------------------------------------

def _blockdiag(att, heads, hid):
    """[heads, hid] -> [heads*hid, heads] block diagonal."""
    out = np.zeros((heads * hid, max(heads, 1)), np.float32)
    for h in range(heads):
        out[h * hid:(h + 1) * hid, h] = att[h]
    return out


def prepare_inputs(inputs, sched: Schedule):
    x = np.asarray(inputs["x"], np.float32)
    ei = np.asarray(inputs["edge_index"])
    W1 = np.asarray(inputs["W1"], np.float32)
    as1 = np.asarray(inputs["att_src1"], np.float32)
    ad1 = np.asarray(inputs["att_dst1"], np.float32)
    W2 = np.asarray(inputs["W2"], np.float32)
    as2 = np.asarray(inputs["att_src2"], np.float32)
    ad2 = np.asarray(inputs["att_dst2"], np.float32)

    N, IN = x.shape
    TR = sched.table_rows
    xp = np.zeros((TR, IN), np.float32)
    xp[:N] = x
    xTb = np.ascontiguousarray(
        xp.T.reshape(2, P, TR).transpose(1, 0, 2)).astype(BF)
    W1b = np.ascontiguousarray(W1.reshape(2, P, 512).transpose(1, 0, 2)).astype(BF)
    W1Tb = np.ascontiguousarray(
        W1.T.reshape(4, P, 256).transpose(1, 0, 2)).astype(BF)
    AcatB = np.ascontiguousarray(
        np.concatenate([_blockdiag(as1, 8, 64), _blockdiag(ad1, 8, 64)], axis=1)
        .reshape(4, P, 16).transpose(1, 0, 2)
    ).astype(BF)
    W2b = np.ascontiguousarray(W2.reshape(4, P, 16).transpose(1, 0, 2)).astype(BF)
    W2Tb = np.ascontiguousarray(W2.T).astype(BF)          # [16, 512]
    att2b = np.concatenate([as2.T, ad2.T], axis=1).astype(BF)  # [16, 2]

    shared = dict(xT=xTb, W1r=W1b, W1Tr=W1Tb, Acat=AcatB, W2r=W2b, W2Tr=W2Tb, att2=att2b)
    maps = []
    for c in range(sched.n_cores):
        pc = sched.per_core[c]
        m = dict(shared)
        m.update(
            idx1A=pc["idx1A"], idx1B=pc["idx1B"], idx2A=pc["idx2A"],
            idx2B=pc["idx2B"], dstloc=pc["dstloc"], dstlocT=pc["dstlocT"],
        )
        maps.append(m)
    return maps


_LAST_RESULT = {}


def kernel(**inputs):
    from concourse.bass_utils import run_bass_kernel_spmd

    x = np.asarray(inputs["x"], np.float32)
    ei = np.asarray(inputs["edge_index"], np.int64)
    N = x.shape[0]
    n_cores = 8
    loops = np.arange(N, dtype=np.int64)
    src = np.concatenate([ei[0], loops])
    dst = np.concatenate([ei[1], loops])

    sched = Schedule(src, dst, N, n_cores)
    phase = os.environ.get("GAT_PHASE", "full")
    nc = build_program(sched, n_cores, phase=phase)
    in_maps = prepare_inputs(inputs, sched)

    trace = bool(int(os.environ.get("GAT_TRACE", "0")))
    res = run_bass_kernel_spmd(
        nc, in_maps, core_ids=list(range(n_cores)), trace=trace,
    )
    _LAST_RESULT["res"] = res

    out = np.zeros((N, 16), np.float32)
    for c in range(n_cores):
        sh = res.results[c]["out_shard"]
        n0 = c * sched.npc
        out[n0:n0 + sched.npc] = sh[: sched.npc]
    return out



# revision 28
# speedup vs baseline: 1.4620x; 1.1181x over previous
"""2-layer GAT (GATConv x2 + log_softmax) on 8 Trainium2 NeuronCores.

Strategy (dst-sharded message passing):
  - Nodes are sharded contiguously across 8 cores (6250 each); every edge is
    owned by the core owning its dst node.  Edges are grouped by dst tile
    (128 dst nodes), split into A/B halves by src id (so gather indices fit
    int16), padded to 128-edge blocks with a cross-core-uniform schedule so
    all 8 cores run one SPMD program.
  - Layer-1 node phase is replicated: every core computes h = x@W1 (bf16,
    fp32 accum) for ALL nodes and writes a gather table
    [h(512) | a_src(8) | a_dst(8) | pad] bf16 per node.
  - Edge phase per 128-edge block: dma_gather rows by src, build a 0/1
    selection matrix SelT[e,d] = (dst_local[e] == d) on DVE, per-head
    weight multiply, then PE matmul SelT.T @ M accumulates the segment sum
    (and the softmax denominator) in PSUM per dst tile.
  - Scores: exp(leaky_relu(a_src[src] + a_dst[dst])) with a_dst gathered
    from a per-core table; softmax normalization is applied per dst tile
    after aggregation (alpha = w/denom pulled out of the edge sum).
  - Layer 2 (1 head, 16 ch) reuses the same block structure; the small
    z-table is exchanged with an AllGather collective.
"""
import os
import math
import numpy as np
import ml_dtypes

import concourse.bass as bass
import concourse.mybir as mybir
import concourse.tile as tile
import concourse.bacc as bacc
from concourse.masks import make_identity
from concourse.library_config import mlp

BF = ml_dtypes.bfloat16
dt = mybir.dt
AF = mybir.ActivationFunctionType
ALU = mybir.AluOpType

P = 128
ROW1 = 640     # table1 cols (bf16): [h 512 | a_src 8 | a_dst 8 | pad]
ROWZ = 128     # z-table cols (bf16): [z 16 | a_src2 1 | a_dst2 1 | pad]
BLKCAP = 40    # max blocks per gather group


# ----------------------------------------------------------------------------
# host-side schedule construction
# ----------------------------------------------------------------------------

def _wrap_idx(vals, slots):
    """Pad `vals` with 0 to `slots`, wrap into [128, slots/16] int16 layout."""
    v = np.zeros(slots, np.int64)
    v[: len(vals)] = vals
    a = v.reshape(-1, 16).T  # [16, slots/16]
    return np.tile(a, (8, 1)).astype(np.int16)


class Schedule:
    """Cross-core-uniform block schedule + per-core index arrays."""

    def __init__(self, src, dst, n_nodes, n_cores, force_split=None):
        self.n_nodes = n_nodes
        self.n_cores = n_cores
        self.npc = n_nodes // n_cores                 # real nodes per core
        self.nt = (self.npc + P - 1) // P             # dst tiles per core
        self.npcp = self.nt * P                       # padded nodes per core
        self.ntot_p = ((n_nodes + P - 1) // P) * P if n_cores == 1 else None
        # padded global table rows (node-id indexed)
        self.table_rows = ((n_nodes + P - 1) // P) * P
        self.table_rows = max(self.table_rows, self.npcp * n_cores)
        self.zrows = self.npcp * n_cores              # z-table rows (zid indexed)

        # split for int16 gathers: src <= SPLIT1-1 -> table A half;
        # zid(src) <= 32767 must also hold.
        if force_split is not None:
            self.split1 = force_split
        elif self.table_rows <= 32768 and self.zrows <= 32768:
            self.split1 = self.table_rows  # no B half
        else:
            # largest s with s-1 <= 32767 and zid(s-1) <= 32767
            s = min(32768, self.n_nodes)
            while s > 0:
                n = s - 1
                zid = (n // self.npc) * self.npcp + (n % self.npc)
                if zid <= 32767:
                    break
                s -= 1
            self.split1 = s
        self.zsplit = ((self.split1 - 1) // self.npc) * self.npcp + (
            (self.split1 - 1) % self.npc
        ) + 1 if self.split1 < self.table_rows else self.zrows

        core = dst // self.npc
        loc = dst - core * self.npc
        t = loc // P
        dloc = loc % P
        isB = src >= self.split1

        nc_, nt_ = n_cores, self.nt
        # counts[core, tile, {A,B}]
        key = (core * nt_ + t) * 2 + isB
        cnt = np.bincount(key, minlength=nc_ * nt_ * 2).reshape(nc_, nt_, 2)
        mx = cnt.max(axis=0)                            # [nt, 2]
        self.Ablk = np.ceil(mx[:, 0] / P).astype(int)
        self.Bblk = np.ceil(mx[:, 1] / P).astype(int)
        self.TBlk = self.Ablk + self.Bblk

        # groups: consecutive tiles, sum(TBlk) <= BLKCAP
        self.groups = []
        cur, acc = [], 0
        for ti in range(nt_):
            tb = int(self.TBlk[ti])
            if cur and acc + tb > BLKCAP:
                self.groups.append(cur)
                cur, acc = [], 0
            cur.append(ti)
            acc += tb
        if cur:
            self.groups.append(cur)

        # canonical block order & per-tile positions within group buffers
        # group buffer layout: [A-blocks of each tile in order, then B-blocks]
        self.g_ablk = []   # per group: total A blocks
        self.g_tblk = []   # per group: total blocks
        self.tile_apos = {}  # tile -> in-group A block offset
        self.tile_bpos = {}  # tile -> in-group block offset of its B blocks
        self.g_base = []     # per group: global block offset
        nblocks = 0
        for g, tl in enumerate(self.groups):
            ga = int(sum(self.Ablk[ti] for ti in tl))
            gt = int(sum(self.TBlk[ti] for ti in tl))
            self.g_ablk.append(ga)
            self.g_tblk.append(gt)
            ao = 0
            bo = ga
            for ti in tl:
                self.tile_apos[ti] = ao
                self.tile_bpos[ti] = bo
                ao += int(self.Ablk[ti])
                bo += int(self.Bblk[ti])
            self.g_base.append(nblocks)
            nblocks += gt
        self.nblocks = nblocks

        self.maxtb = int(self.TBlk.max())

        # per-core arrays
        # order edges by (core, tile, isB) stably
        order = np.lexsort((isB, t, core))
        self.per_core = []
        for c in range(nc_):
            m0 = order[core[order] == c]
            idx1A_cols, idx1B_cols, idx2A_cols, idx2B_cols = [], [], [], []
            dstloc = np.full((P, nblocks), 999.0, np.float32)
            for g, tl in enumerate(self.groups):
                a_src_l, b_src_l = [], []
                dl_A, dl_B = [], []
                for ti in tl:
                    e = m0[t[m0] == ti]
                    eA = e[~isB[e]]
                    eB = e[isB[e]]
                    nA = int(self.Ablk[ti]) * P
                    nB = int(self.Bblk[ti]) * P
                    sA = np.zeros(nA, np.int64)
                    sA[: len(eA)] = src[eA]
                    sB = np.zeros(nB, np.int64)
                    sB[: len(eB)] = src[eB] - self.split1
                    lA = np.full(nA, 999.0, np.float32)
                    lA[: len(eA)] = dloc[eA]
                    lB = np.full(nB, 999.0, np.float32)
                    lB[: len(eB)] = dloc[eB]
                    a_src_l.append(sA)
                    b_src_l.append(sB)
                    dl_A.append(lA)
                    dl_B.append(lB)
                gsA = np.concatenate(a_src_l) if a_src_l else np.zeros(0, np.int64)
                gsB = np.concatenate(b_src_l) if b_src_l else np.zeros(0, np.int64)
                gdl = np.concatenate(dl_A + dl_B) if (dl_A or dl_B) else np.zeros(0, np.float32)
                # L2 indices: zid mapping of global src
                def zid_of(v):
                    vv = np.asarray(v, np.int64)
                    return (vv // self.npc) * self.npcp + (vv % self.npc)
                g2A = zid_of(gsA)                       # gsA holds global src (pads=0)
                g2B = zid_of(gsB + self.split1) - self.zsplit
                idx1A_cols.append(_wrap_idx(gsA, len(gsA)))
                idx1B_cols.append(_wrap_idx(gsB, len(gsB)))
                idx2A_cols.append(_wrap_idx(g2A, len(g2A)))
                idx2B_cols.append(_wrap_idx(g2B, len(g2B)))
                gb = self.g_base[g]
                dstloc[:, gb : gb + self.g_tblk[g]] = gdl.reshape(-1, P).T
            cat = lambda ls: (
                np.concatenate(ls, axis=1) if ls and sum(x.shape[1] for x in ls) else np.zeros((P, 1), np.int16)
            )
            # partition-replicated transposed dstloc (pad -> 512, exact in bf16)
            dl = dstloc.T.copy()                       # [nblocks, P(edge)]
            dl[dl == 999.0] = 512.0
            dstlocT = np.broadcast_to(
                dl[None, :, :], (P, nblocks, P)
            ).astype(BF).copy()
            self.per_core.append(
                dict(
                    idx1A=cat(idx1A_cols), idx1B=cat(idx1B_cols),
                    idx2A=cat(idx2A_cols), idx2B=cat(idx2B_cols),
                    dstloc=dstloc, dstlocT=dstlocT,
                )
            )
        # column offsets per group in the concatenated idx arrays
        self.gA_coloff, self.gB_coloff = [], []
        a = b = 0
        for g in range(len(self.groups)):
            self.gA_coloff.append(a)
            self.gB_coloff.append(b)
            a += (self.g_ablk[g] * P) // 16
            b += ((self.g_tblk[g] - self.g_ablk[g]) * P) // 16
        self.totA_cols = max(a, 1)
        self.totB_cols = max(b, 1)


# ----------------------------------------------------------------------------
# device program
# ----------------------------------------------------------------------------

def build_program(sched: Schedule, n_cores: int, phase: str = 'full'):
    """Build the SPMD Bass/Tile program for the given schedule."""
    nc = bacc.Bacc(None, target_bir_lowering=False, debug=True, num_devices=n_cores)

    TR = sched.table_rows
    ZR = sched.zrows
    NT = sched.nt
    NPC, NPCP = sched.npc, sched.npcp
    NODE_TILES = TR // P

    # ---- inputs -------------------------------------------------------------
    xT = nc.dram_tensor("xT", [P, 2, TR], dt.float8e4, kind="ExternalInput")
    W1r = nc.dram_tensor("W1r", [P, 2, 512], dt.float8e4, kind="ExternalInput")
    W1Tr = nc.dram_tensor("W1Tr", [P, 4, 256], dt.bfloat16, kind="ExternalInput")
    Acat = nc.dram_tensor("Acat", [P, 4, 16], dt.bfloat16, kind="ExternalInput")
    W2r = nc.dram_tensor("W2r", [P, 4, 16], dt.bfloat16, kind="ExternalInput")
    W2Tr = nc.dram_tensor("W2Tr", [16, 512], dt.bfloat16, kind="ExternalInput")
    att2 = nc.dram_tensor("att2", [16, 2], dt.bfloat16, kind="ExternalInput")
    idx1A = nc.dram_tensor("idx1A", [P, sched.totA_cols], dt.int16, kind="ExternalInput")
    idx1B = nc.dram_tensor("idx1B", [P, sched.totB_cols], dt.int16, kind="ExternalInput")
    idx2A = nc.dram_tensor("idx2A", [P, sched.totA_cols], dt.int16, kind="ExternalInput")
    idx2B = nc.dram_tensor("idx2B", [P, sched.totB_cols], dt.int16, kind="ExternalInput")
    dstlocr = nc.dram_tensor("dstloc", [P, sched.nblocks], dt.float32, kind="ExternalInput")
    dstlocTr = nc.dram_tensor("dstlocT", [P, sched.nblocks, P], dt.bfloat16, kind="ExternalInput")
    out_shard = nc.dram_tensor("out_shard", [NPCP, 16], dt.float32, kind="ExternalOutput")

    with tile.TileContext(nc) as tc:
        nc.gpsimd.load_library(mlp)
        with (
            tc.tile_pool(name="dram", bufs=1, space="DRAM") as dram,
            tc.tile_pool(name="const", bufs=1) as cpool,
        ):
            table1 = dram.tile([TR, ROW1], dt.bfloat16)
            adst_own = dram.tile([NPCP, 8], dt.bfloat16)
            cc_in = dram.tile([NPCP, ROWZ], dt.bfloat16)
            cc_out = dram.tile([ZR, ROWZ], dt.bfloat16,
                               addr_space=("Shared" if n_cores > 1 else "Local"))

            # ---- constants -------------------------------------------------
            iota_i = cpool.tile([P, P], dt.int32)
            nc.gpsimd.iota(iota_i[:], pattern=[[1, P]], base=0, channel_multiplier=0)
            iota_bf = cpool.tile([P, P], dt.bfloat16)
            nc.vector.tensor_copy(iota_bf[:], iota_i[:])
            iota_ci = cpool.tile([P, 1], dt.int32)
            nc.gpsimd.iota(iota_ci[:], pattern=[[0, 1]], base=0, channel_multiplier=1)
            iota_cf = cpool.tile([P, 1], dt.float32)
            nc.vector.tensor_copy(iota_cf[:], iota_ci[:])
            ident = cpool.tile([P, P], dt.bfloat16)
            make_identity(nc, ident[:])

            W1s = cpool.tile([P, 2, 512], dt.float8e4)
            nc.sync.dma_start(W1s[:], W1r[:])
            W1Ts = cpool.tile([P, 4, 256], dt.bfloat16)
            nc.sync.dma_start(W1Ts[:], W1Tr[:])
            Acats = cpool.tile([P, 4, 16], dt.bfloat16)
            nc.sync.dma_start(Acats[:], Acat[:])
            W2Ts = cpool.tile([16, 512], dt.bfloat16)
            nc.sync.dma_start(W2Ts[:], W2Tr[:])
            att2s = cpool.tile([16, 2], dt.bfloat16)
            nc.sync.dma_start(att2s[:], att2[:])
            dstloc_s = cpool.tile([P, sched.nblocks], dt.float32)
            nc.sync.dma_start(dstloc_s[:], dstlocr[:])
            i1A = cpool.tile([P, sched.totA_cols], dt.int16)
            nc.sync.dma_start(i1A[:], idx1A[:])
            i1B = cpool.tile([P, sched.totB_cols], dt.int16)
            nc.sync.dma_start(i1B[:], idx1B[:])
            i2A = cpool.tile([P, sched.totA_cols], dt.int16)
            nc.sync.dma_start(i2A[:], idx2A[:])
            i2B = cpool.tile([P, sched.totB_cols], dt.int16)
            nc.sync.dma_start(i2B[:], idx2B[:])
            adst_all = cpool.tile([P, NT, 8], dt.bfloat16)
            azdst_all = cpool.tile([P, NT, 1], dt.bfloat16)

            # fused attention weights: wcat = W1 @ [Asrc|Adst]  -> [2,128,16]
            wcat = cpool.tile([P, 2, 16], dt.bfloat16)
            wcatq = cpool.tile([P, 2, 16], dt.float8e4)
            wz = cpool.tile([P, 4, 18], dt.bfloat16)   # [W2 | wcat2] per 128-chunk
            W2s = cpool.tile([P, 4, 16], dt.bfloat16)
            nc.sync.dma_start(W2s[:], W2r[:])
            with tc.tile_pool(name="p0ps", bufs=2, space="PSUM") as p0ps:
                for i in range(2):
                    ps = p0ps.tile([P, 16], dt.float32, tag="wc")
                    for j in range(4):
                        nc.tensor.matmul(
                            ps[:], W1Ts[:, j, i * P:(i + 1) * P], Acats[:, j, :],
                            start=(j == 0), stop=(j == 3),
                        )
                    nc.vector.tensor_copy(wcat[:, i, :], ps[:])
                    nc.vector.tensor_scalar(
                        out=wcatq[:, i, :], in0=ps[:], scalar1=16.0,
                        scalar2=None, op0=ALU.mult,
                    )
                for cch in range(4):
                    ps2 = p0ps.tile([P, 2], dt.float32, tag="wz")
                    nc.tensor.matmul(
                        ps2[:], W2Ts[:, cch * P:(cch + 1) * P], att2s[:],
                        start=True, stop=True,
                    )
                    nc.vector.tensor_copy(wz[:, cch, 16:18], ps2[:])
                    nc.vector.tensor_copy(wz[:, cch, 0:16], W2s[:, cch, :])

            # ---- P1: replicated node phase --------------------------------
            XB = 4  # node tiles per x load
            with (
                tc.tile_pool(name="p1sb", bufs=3) as p1sb,
                tc.tile_pool(name="p1ps", bufs=2, space="PSUM") as p1ps,
            ):
                DR = mybir.MatmulPerfMode.DoubleRow
                for tq in range(0, NODE_TILES, XB):
                    nb = min(XB, NODE_TILES - tq)
                    xt = p1sb.tile([P, 2, nb * P], dt.float8e4, tag="xt")
                    nc.sync.dma_start(xt[:], xT[:, :, tq * P: tq * P + nb * P])
                    for u in range(nb):
                        ph = p1ps.tile([P, 512], dt.float32, tag="ph")
                        pa = p1ps.tile([P, 16], dt.float32, tag="pa")
                        lhs = xt[:, :, u * P:(u + 1) * P]
                        nc.tensor.matmul(ph[:], lhs, W1s[:], perf_mode=DR,
                                         start=True, stop=True)
                        nc.tensor.matmul(pa[:], lhs, wcatq[:], perf_mode=DR,
                                         start=True, stop=True)
                        rowt = p1sb.tile([P, ROW1], dt.bfloat16, tag="rowt")
                        nc.scalar.activation(rowt[:, 0:512], ph[:], AF.Copy,
                                             scale=0.0625)
                        nc.vector.tensor_scalar(
                            out=rowt[:, 512:528], in0=pa[:], scalar1=0.0625,
                            scalar2=None, op0=ALU.mult,
                        )
                        nc.sync.dma_start(
                            table1[(tq + u) * P:(tq + u + 1) * P, :], rowt[:]
                        )

            # ---- P1.5: per-core a_dst table (SBUF, tile-major) ------------
            if phase not in ("p1",):
                rbase = nc.sync.partition_id() * NPC
                nc.sync.dma_start(
                    adst_own[:, :],
                    table1[bass.ds(rbase, NPCP), 520:528],
                )
                nc.sync.dma_start(
                    adst_all[:],
                    adst_own[:].rearrange("(t p) c -> p t c", p=P),
                )

            # ---- edge phase helper ----------------------------------------
            def edge_phase(layer):
                """layer 1: table1 gathers, 8 heads; layer 2: z-table, 1 head."""
                sub = os.environ.get("GAT_L1SUB", "full")
                if layer == 1:
                    g_src_tab_A, g_src_tab_B = table1, table1[sched.split1:, :]
                    g_elem, g_row = ROW1, ROW1
                    iA, iB = i1A, i1B
                    NH = 8
                    adst_t = adst_all
                else:
                    g_src_tab_A, g_src_tab_B = cc_out, cc_out[sched.zsplit:, :]
                    g_elem, g_row = ROWZ, ROWZ
                    iA, iB = i2A, i2B
                    NH = 1
                    adst_t = azdst_all
                MAXTB = sched.maxtb
                with (
                    tc.tile_pool(name=f"ed{layer}", bufs=2) as ep,
                    tc.tile_pool(name=f"sl{layer}", bufs=2) as sp,
                    tc.tile_pool(name=f"eb{layer}", bufs=3) as bp,
                    tc.tile_pool(name=f"ep{layer}", bufs=2, space="PSUM") as pp,
                    tc.tile_pool(name=f"eo{layer}", bufs=1, space="PSUM") as po,
                    tc.tile_pool(name=f"eq{layer}", bufs=2, space="PSUM") as pq,
                ):
                    for g, tl in enumerate(sched.groups):
                        GB = sched.g_tblk[g]
                        GA = sched.g_ablk[g]
                        nA, nB_ = GA * P, (GB - GA) * P
                        gb = sched.g_base[g]
                        hg = ep.tile([P, GB, g_elem], dt.bfloat16, tag="hg")
                        if nA:
                            nc.gpsimd.dma_gather(
                                hg[:, 0:GA, :], g_src_tab_A[:],
                                iA[:, sched.gA_coloff[g]: sched.gA_coloff[g] + nA // 16],
                                nA, nA, g_elem, single_packet=False,
                            )
                        if nB_:
                            nc.gpsimd.dma_gather(
                                hg[:, GA:GB, :], g_src_tab_B,
                                iB[:, sched.gB_coloff[g]: sched.gB_coloff[g] + nB_ // 16],
                                nB_, nB_, g_elem, elem_step=g_row,
                                single_packet=False,
                            )
                        # transposed dstloc rows for this group (HWDGE stream)
                        dlT = ep.tile([P, GB, P], dt.bfloat16, tag="dlT")
                        nc.sync.dma_start(dlT[:], dstlocTr[:, gb:gb + GB, :])
                        if sub == "gather":
                            continue
                        for ti in tl:
                            ab, bb = int(sched.Ablk[ti]), int(sched.Bblk[ti])
                            tb = ab + bb
                            if tb == 0:
                                continue
                            apos, bpos = sched.tile_apos[ti], sched.tile_bpos[ti]
                            # (in-buffer block, w-slot) pairs for this tile
                            spans = []
                            if ab:
                                spans.append((apos, ab, 0))
                            if bb:
                                spans.append((bpos, bb, ab))
                            # selection matrices (both orientations, DVE only,
                            # one batched op per span) + a_dst broadcast:
                            # aDs[e, j, h] = a_dst[dstloc[e], h]
                            selS = sp.tile([P, MAXTB, P], dt.bfloat16, tag="selS")
                            sel2S = sp.tile([P, MAXTB, P], dt.bfloat16, tag="sel2S")
                            aDs = pq.tile([P, MAXTB, NH], dt.float32, tag="aDs")
                            for (o, n, wo) in spans:
                                gcol = gb + o
                                nc.vector.tensor_tensor(
                                    out=selS[:, wo:wo + n, :],
                                    in0=iota_bf[:].rearrange(
                                        "p (a b) -> p a b", a=1
                                    ).to_broadcast([P, n, P]),
                                    in1=dstloc_s[:, gcol:gcol + n].rearrange(
                                        "p (a b) -> p a b", b=1
                                    ).to_broadcast([P, n, P]),
                                    op=ALU.is_equal,
                                )
                                nc.vector.tensor_scalar(
                                    out=sel2S[:, wo:wo + n, :],
                                    in0=dlT[:, o:o + n, :],
                                    scalar1=iota_cf[:, 0:1],
                                    scalar2=None, op0=ALU.is_equal,
                                )
                                for k in range(n):
                                    j = wo + k
                                    nc.tensor.matmul(
                                        aDs[:, j, :], sel2S[:, j, :], adst_t[:, ti, :],
                                        start=True, stop=True,
                                    )
                            wt = bp.tile([P, tb * NH], dt.float32, tag="wt")
                            for (o, n, wo) in spans:
                                st = bp.tile([P, n * NH], dt.float32, tag="st")
                                if layer == 1:
                                    a_s = hg[:, o:o + n, 512:520]
                                else:
                                    a_s = hg[:, o:o + n, 17:18]
                                st3 = st[:].rearrange("p (n k) -> p n k", k=NH)
                                nc.vector.tensor_tensor(
                                    out=st3, in0=a_s, in1=aDs[:, wo:wo + n, :], op=ALU.add
                                )
                                lk = bp.tile([P, n * NH], dt.float32, tag="lk")
                                nc.vector.scalar_tensor_tensor(
                                    lk[:], st[:], 0.2, st[:], ALU.mult, ALU.max
                                )
                                nc.scalar.activation(
                                    wt[:, wo * NH:(wo + n) * NH], lk[:], AF.Exp
                                )
                            if layer == 1:
                                wbf = bp.tile([P, tb * NH], dt.bfloat16, tag="wbf")
                                nc.vector.tensor_copy(wbf[:], wt[:])
                            if sub == "score":
                                continue
                            if layer == 1:
                                psum_o = pp.tile([P, 512], dt.float32, tag="psO")
                                psum_d = po.tile([P, 8], dt.float32, tag="psD")
                            else:
                                psum_o = pp.tile([P, 17], dt.float32, tag="psO")
                                psum_d = None
                            j = 0
                            for (o, n, wo) in spans:
                                for k in range(n):
                                    blk = o + k
                                    jj = wo + k
                                    first, last = (j == 0), (j == tb - 1)
                                    ws = jj * NH
                                    if layer == 1:
                                        M = bp.tile([P, 512], dt.bfloat16, tag="M")
                                        wbc = wbf[:, ws:ws + 8].rearrange(
                                            "p (a b) -> p a b", b=1
                                        ).to_broadcast([P, 8, 64])
                                        nc.vector.tensor_tensor(
                                            out=M[:].rearrange("p (a b) -> p a b", b=64),
                                            in0=hg[:, blk, 0:512].rearrange("p (a b) -> p a b", b=64),
                                            in1=wbc,
                                            op=ALU.mult,
                                        )
                                        nc.tensor.matmul(psum_o[:], selS[:, jj, :], M[:], start=first, stop=last)
                                        nc.tensor.matmul(
                                            psum_d[:], selS[:, jj, :], wbf[:, ws:ws + 8],
                                            start=first, stop=last,
                                        )
                                    else:
                                        # cols 0:16 = z*w, col 16 = w (z-row col 16
                                        # holds 1.0) -> denominator rides along
                                        M = bp.tile([P, 17], dt.bfloat16, tag="M")
                                        nc.vector.tensor_scalar_mul(
                                            M[:], hg[:, blk, 0:17], wt[:, ws:ws + 1]
                                        )
                                        nc.tensor.matmul(psum_o[:], selS[:, jj, :], M[:], start=first, stop=last)
                                    j += 1
                            if sub == "blocks":
                                continue
                            close_tile(layer, ti, psum_o, psum_d, bp, pp, po)

            # ---- tile close -------------------------------------------------
            def close_tile(layer, ti, psum_o, psum_d, bp, pp, pq):
                if layer == 1:
                    r = bp.tile([P, 8], dt.float32, tag="r")
                    nc.vector.reciprocal(r[:], psum_d[:])
                    o1 = bp.tile([P, 512], dt.bfloat16, tag="o1")
                    o13 = o1[:].rearrange("p (a b) -> p a b", b=64)
                    rbc = r[:].rearrange("p (a b) -> p a b", b=1).to_broadcast([P, 8, 64])
                    nc.vector.tensor_tensor(
                        out=o13,
                        in0=psum_o[:].rearrange("p (a b) -> p a b", b=64),
                        in1=rbc, op=ALU.mult,
                    )
                    # elu: h2 = max(o1,0) + exp(min(o1,0)) - 1
                    u = bp.tile([P, 512], dt.bfloat16, tag="u")
                    nc.vector.tensor_scalar_min(u[:], o1[:], 0.0)
                    e1 = bp.tile([P, 512], dt.bfloat16, tag="e1")
                    nc.scalar.activation(e1[:], u[:], AF.Exp)
                    rv = bp.tile([P, 512], dt.bfloat16, tag="rv")
                    nc.vector.tensor_scalar_max(rv[:], o1[:], 0.0)
                    h2 = bp.tile([P, 512], dt.bfloat16, tag="h2")
                    nc.vector.scalar_tensor_tensor(
                        h2[:], e1[:], -1.0, rv[:], ALU.add, ALU.add
                    )
                    # transpose h2 -> z matmuls
                    pz = pq.tile([P, 18], dt.float32, tag="psZ")
                    for c in range(4):
                        ptr = pp.tile([P, P], dt.bfloat16, tag="psT")
                        nc.tensor.transpose(ptr[:], h2[:, c * P:(c + 1) * P], ident[:])
                        h2T = bp.tile([P, P], dt.bfloat16, tag="h2T")
                        nc.scalar.copy(h2T[:], ptr[:])
                        nc.tensor.matmul(pz[:], h2T[:], wz[:, c, :], start=(c == 0), stop=(c == 3))
                    # z-row layout: [z 16 | 1.0 | a_src2 | a_dst2 | junk]
                    zrow = bp.tile([P, ROWZ], dt.bfloat16, tag="zrow")
                    nc.vector.tensor_copy(zrow[:, 0:16], pz[:, 0:16])
                    nc.vector.memset(zrow[:, 16:17], 1.0)
                    nc.vector.tensor_copy(zrow[:, 17:19], pz[:, 16:18])
                    nc.sync.dma_start(cc_in[ti * P:(ti + 1) * P, :], zrow[:])
                else:
                    r2 = bp.tile([P, 1], dt.float32, tag="r2")
                    nc.vector.reciprocal(r2[:], psum_o[:, 16:17])
                    o2 = bp.tile([P, 16], dt.float32, tag="o2")
                    nc.vector.tensor_scalar_mul(o2[:], psum_o[:, 0:16], r2[:, 0:1])
                    mx = bp.tile([P, 1], dt.float32, tag="mx")
                    nc.vector.tensor_reduce(mx[:], o2[:], axis=mybir.AxisListType.X, op=ALU.max)
                    o2m = bp.tile([P, 16], dt.float32, tag="o2m")
                    nc.vector.tensor_scalar_sub(o2m[:], o2[:], mx[:, 0:1])
                    ex = bp.tile([P, 16], dt.float32, tag="ex")
                    ssum = bp.tile([P, 1], dt.float32, tag="ssum")
                    nc.scalar.activation(ex[:], o2m[:], AF.Exp, accum_out=ssum[:])
                    lse = bp.tile([P, 1], dt.float32, tag="lse")
                    nc.scalar.activation(lse[:], ssum[:], AF.Ln)
                    res = bp.tile([P, 16], dt.float32, tag="res")
                    nc.vector.tensor_scalar_sub(res[:], o2m[:], lse[:, 0:1])
                    nc.sync.dma_start(out_shard[ti * P:(ti + 1) * P, :], res[:])

            if phase not in ("p1", "p15"):
                edge_phase(1)

            if phase in ("cc", "full"):
                # ---- z-table exchange -------------------------------------
                if n_cores == 1:
                    nc.sync.dma_start(cc_out[:, :], cc_in[:, :])
                else:
                    nc.gpsimd.collective_compute(
                        "AllGather", ALU.bypass,
                        ins=[cc_in[:]], outs=[cc_out[:]],
                        replica_groups=[list(range(n_cores))],
                    )

            if phase == "full":
                nc.sync.dma_start(
                    azdst_all[:],
                    cc_in[:, 18:19].rearrange("(t p) c -> p t c", p=P),
                )
                edge_phase(2)

    nc.compile()
    return nc


# ----------------------------------------------------------------------------
# host entry
# ----------------------------------------------------------------------------

def _blockdiag(att, heads, hid):
    """[heads, hid] -> [heads*hid, heads] block diagonal."""
    out = np.zeros((heads * hid, max(heads, 1)), np.float32)
    for h in range(heads):
        out[h * hid:(h + 1) * hid, h] = att[h]
    return out


def prepare_inputs(inputs, sched: Schedule):
    x = np.asarray(inputs["x"], np.float32)
    ei = np.asarray(inputs["edge_index"])
    W1 = np.asarray(inputs["W1"], np.float32)
    as1 = np.asarray(inputs["att_src1"], np.float32)
    ad1 = np.asarray(inputs["att_dst1"], np.float32)
    W2 = np.asarray(inputs["W2"], np.float32)
    as2 = np.asarray(inputs["att_src2"], np.float32)
    ad2 = np.asarray(inputs["att_dst2"], np.float32)

    N, IN = x.shape
    TR = sched.table_rows
    xp = np.zeros((TR, IN), np.float32)
    xp[:N] = x
    F8 = ml_dtypes.float8_e4m3
    xTb = np.ascontiguousarray(
        xp.T.reshape(2, P, TR).transpose(1, 0, 2)).astype(F8)
    W1b = np.ascontiguousarray(
        (W1 * 16.0).reshape(2, P, 512).transpose(1, 0, 2)).astype(F8)
    W1Tb = np.ascontiguousarray(
        W1.T.reshape(4, P, 256).transpose(1, 0, 2)).astype(BF)
    AcatB = np.ascontiguousarray(
        np.concatenate([_blockdiag(as1, 8, 64), _blockdiag(ad1, 8, 64)], axis=1)
        .reshape(4, P, 16).transpose(1, 0, 2)
    ).astype(BF)
    W2b = np.ascontiguousarray(W2.reshape(4, P, 16).transpose(1, 0, 2)).astype(BF)
    W2Tb = np.ascontiguousarray(W2.T).astype(BF)          # [16, 512]
    att2b = np.concatenate([as2.T, ad2.T], axis=1).astype(BF)  # [16, 2]

    shared = dict(xT=xTb, W1r=W1b, W1Tr=W1Tb, Acat=AcatB, W2r=W2b, W2Tr=W2Tb, att2=att2b)
    maps = []
    for c in range(sched.n_cores):
        pc = sched.per_core[c]
        m = dict(shared)
        m.update(
            idx1A=pc["idx1A"], idx1B=pc["idx1B"], idx2A=pc["idx2A"],
            idx2B=pc["idx2B"], dstloc=pc["dstloc"], dstlocT=pc["dstlocT"],
        )
        maps.append(m)
    return maps


_LAST_RESULT = {}


def kernel(**inputs):
    from concourse.bass_utils import run_bass_kernel_spmd

    x = np.asarray(inputs["x"], np.float32)
    ei = np.asarray(inputs["edge_index"], np.int64)
    N = x.shape[0]
    n_cores = 8
    loops = np.arange(N, dtype=np.int64)
    src = np.concatenate([ei[0], loops])
    dst = np.concatenate([ei[1], loops])

    sched = Schedule(src, dst, N, n_cores)
    phase = os.environ.get("GAT_PHASE", "full")
    nc = build_program(sched, n_cores, phase=phase)
    in_maps = prepare_inputs(inputs, sched)

    trace = bool(int(os.environ.get("GAT_TRACE", "0")))
    res = run_bass_kernel_spmd(
        nc, in_maps, core_ids=list(range(n_cores)), trace=trace,
    )
    _LAST_RESULT["res"] = res

    out = np.zeros((N, 16), np.float32)
    for c in range(n_cores):
        sh = res.results[c]["out_shard"]
        n0 = c * sched.npc
        out[n0:n0 + sched.npc] = sh[: sched.npc]
    return out



# revision 37
# speedup vs baseline: 1.6780x; 1.1477x over previous
"""2-layer GAT (GATConv x2 + log_softmax) on 8 Trainium2 NeuronCores.

Strategy (dst-sharded message passing):
  - Nodes are sharded contiguously across 8 cores (6250 each); every edge is
    owned by the core owning its dst node.  Edges are grouped by dst tile
    (128 dst nodes), split into A/B halves by src id (so gather indices fit
    int16), padded to 128-edge blocks with a cross-core-uniform schedule so
    all 8 cores run one SPMD program.
  - Layer-1 node phase is replicated: every core computes h = x@W1 (bf16,
    fp32 accum) for ALL nodes and writes a gather table
    [h(512) | a_src(8) | a_dst(8) | pad] bf16 per node.
  - Edge phase per 128-edge block: dma_gather rows by src, build a 0/1
    selection matrix SelT[e,d] = (dst_local[e] == d) on DVE, per-head
    weight multiply, then PE matmul SelT.T @ M accumulates the segment sum
    (and the softmax denominator) in PSUM per dst tile.
  - Scores: exp(leaky_relu(a_src[src] + a_dst[dst])) with a_dst gathered
    from a per-core table; softmax normalization is applied per dst tile
    after aggregation (alpha = w/denom pulled out of the edge sum).
  - Layer 2 (1 head, 16 ch) reuses the same block structure; the small
    z-table is exchanged with an AllGather collective.
"""
import os
import math
import numpy as np
import ml_dtypes

import concourse.bass as bass
import concourse.mybir as mybir
import concourse.tile as tile
import concourse.bacc as bacc
from concourse.masks import make_identity
from concourse.library_config import mlp

BF = ml_dtypes.bfloat16
dt = mybir.dt
AF = mybir.ActivationFunctionType
ALU = mybir.AluOpType

P = 128
ROW1 = 640     # table1 cols (bf16): [h 512 | a_src 8 | a_dst 8 | pad]
ROWZ = 128     # z-table cols (bf16): [z 16 | a_src2 1 | a_dst2 1 | pad]
BLKCAP = 32    # max blocks per gather group


# ----------------------------------------------------------------------------
# host-side schedule construction
# ----------------------------------------------------------------------------

def _wrap_idx(vals, slots):
    """Pad `vals` with 0 to `slots`, wrap into [128, slots/16] int16 layout."""
    v = np.zeros(slots, np.int64)
    v[: len(vals)] = vals
    a = v.reshape(-1, 16).T  # [16, slots/16]
    return np.tile(a, (8, 1)).astype(np.int16)


class Schedule:
    """Cross-core-uniform block schedule + per-core index arrays."""

    def __init__(self, src, dst, n_nodes, n_cores, force_split=None):
        self.n_nodes = n_nodes
        self.n_cores = n_cores
        self.npc = n_nodes // n_cores                 # real nodes per core
        self.nt = (self.npc + P - 1) // P             # dst tiles per core
        self.npcp = self.nt * P                       # padded nodes per core
        self.ntot_p = ((n_nodes + P - 1) // P) * P if n_cores == 1 else None
        # padded global table rows (node-id indexed)
        self.table_rows = ((n_nodes + P - 1) // P) * P
        self.table_rows = max(self.table_rows, self.npcp * n_cores)
        self.zrows = self.npcp * n_cores              # z-table rows (zid indexed)

        # split for int16 gathers: src <= SPLIT1-1 -> table A half;
        # zid(src) <= 32767 must also hold.
        if force_split is not None:
            self.split1 = force_split
        elif self.table_rows <= 32768 and self.zrows <= 32768:
            self.split1 = self.table_rows  # no B half
        else:
            # largest s with s-1 <= 32767 and zid(s-1) <= 32767
            s = min(32768, self.n_nodes)
            while s > 0:
                n = s - 1
                zid = (n // self.npc) * self.npcp + (n % self.npc)
                if zid <= 32767:
                    break
                s -= 1
            self.split1 = s
        self.zsplit = ((self.split1 - 1) // self.npc) * self.npcp + (
            (self.split1 - 1) % self.npc
        ) + 1 if self.split1 < self.table_rows else self.zrows

        core = dst // self.npc
        loc = dst - core * self.npc
        t = loc // P
        dloc = loc % P
        isB = src >= self.split1

        nc_, nt_ = n_cores, self.nt
        # counts[core, tile, {A,B}]
        key = (core * nt_ + t) * 2 + isB
        cnt = np.bincount(key, minlength=nc_ * nt_ * 2).reshape(nc_, nt_, 2)
        mx = cnt.max(axis=0)                            # [nt, 2]
        self.Ablk = np.ceil(mx[:, 0] / P).astype(int)
        self.Bblk = np.ceil(mx[:, 1] / P).astype(int)
        self.TBlk = self.Ablk + self.Bblk

        # groups: consecutive tiles, sum(TBlk) <= BLKCAP
        self.groups = []
        cur, acc = [], 0
        for ti in range(nt_):
            tb = int(self.TBlk[ti])
            if cur and acc + tb > BLKCAP:
                self.groups.append(cur)
                cur, acc = [], 0
            cur.append(ti)
            acc += tb
        if cur:
            self.groups.append(cur)

        # canonical block order & per-tile positions within group buffers
        # group buffer layout: [A-blocks of each tile in order, then B-blocks]
        self.g_ablk = []   # per group: total A blocks
        self.g_tblk = []   # per group: total blocks
        self.tile_apos = {}  # tile -> in-group A block offset
        self.tile_bpos = {}  # tile -> in-group block offset of its B blocks
        self.g_base = []     # per group: global block offset
        nblocks = 0
        for g, tl in enumerate(self.groups):
            ga = int(sum(self.Ablk[ti] for ti in tl))
            gt = int(sum(self.TBlk[ti] for ti in tl))
            self.g_ablk.append(ga)
            self.g_tblk.append(gt)
            ao = 0
            bo = ga
            for ti in tl:
                self.tile_apos[ti] = ao
                self.tile_bpos[ti] = bo
                ao += int(self.Ablk[ti])
                bo += int(self.Bblk[ti])
            self.g_base.append(nblocks)
            nblocks += gt
        self.nblocks = nblocks

        self.maxtb = int(self.TBlk.max())

        # per-core arrays
        # order edges by (core, tile, isB) stably
        order = np.lexsort((isB, t, core))
        self.per_core = []
        for c in range(nc_):
            m0 = order[core[order] == c]
            idx1A_cols, idx1B_cols, idx2A_cols, idx2B_cols = [], [], [], []
            dstloc = np.full((P, nblocks), 999.0, np.float32)
            for g, tl in enumerate(self.groups):
                a_src_l, b_src_l = [], []
                dl_A, dl_B = [], []
                for ti in tl:
                    e = m0[t[m0] == ti]
                    eA = e[~isB[e]]
                    eB = e[isB[e]]
                    nA = int(self.Ablk[ti]) * P
                    nB = int(self.Bblk[ti]) * P
                    sA = np.zeros(nA, np.int64)
                    sA[: len(eA)] = src[eA]
                    sB = np.zeros(nB, np.int64)
                    sB[: len(eB)] = src[eB] - self.split1
                    lA = np.full(nA, 999.0, np.float32)
                    lA[: len(eA)] = dloc[eA]
                    lB = np.full(nB, 999.0, np.float32)
                    lB[: len(eB)] = dloc[eB]
                    a_src_l.append(sA)
                    b_src_l.append(sB)
                    dl_A.append(lA)
                    dl_B.append(lB)
                gsA = np.concatenate(a_src_l) if a_src_l else np.zeros(0, np.int64)
                gsB = np.concatenate(b_src_l) if b_src_l else np.zeros(0, np.int64)
                gdl = np.concatenate(dl_A + dl_B) if (dl_A or dl_B) else np.zeros(0, np.float32)
                # L2 indices: zid mapping of global src
                def zid_of(v):
                    vv = np.asarray(v, np.int64)
                    return (vv // self.npc) * self.npcp + (vv % self.npc)
                g2A = zid_of(gsA)                       # gsA holds global src (pads=0)
                g2B = zid_of(gsB + self.split1) - self.zsplit
                idx1A_cols.append(_wrap_idx(gsA, len(gsA)))
                idx1B_cols.append(_wrap_idx(gsB, len(gsB)))
                idx2A_cols.append(_wrap_idx(g2A, len(g2A)))
                idx2B_cols.append(_wrap_idx(g2B, len(g2B)))
                gb = self.g_base[g]
                dstloc[:, gb : gb + self.g_tblk[g]] = gdl.reshape(-1, P).T
            cat = lambda ls: (
                np.concatenate(ls, axis=1) if ls and sum(x.shape[1] for x in ls) else np.zeros((P, 1), np.int16)
            )
            # partition-replicated transposed dstloc (pad -> 512, exact in bf16)
            dl = dstloc.T.copy()                       # [nblocks, P(edge)]
            dl[dl == 999.0] = 512.0
            dstlocT = np.broadcast_to(
                dl[None, :, :], (P, nblocks, P)
            ).astype(BF).copy()
            self.per_core.append(
                dict(
                    idx1A=cat(idx1A_cols), idx1B=cat(idx1B_cols),
                    idx2A=cat(idx2A_cols), idx2B=cat(idx2B_cols),
                    dstloc=dstloc, dstlocT=dstlocT,
                )
            )
        # column offsets per group in the concatenated idx arrays
        self.gA_coloff, self.gB_coloff = [], []
        a = b = 0
        for g in range(len(self.groups)):
            self.gA_coloff.append(a)
            self.gB_coloff.append(b)
            a += (self.g_ablk[g] * P) // 16
            b += ((self.g_tblk[g] - self.g_ablk[g]) * P) // 16
        self.totA_cols = max(a, 1)
        self.totB_cols = max(b, 1)


# ----------------------------------------------------------------------------
# device program
# ----------------------------------------------------------------------------

def build_program(sched: Schedule, n_cores: int, phase: str = 'full'):
    """Build the SPMD Bass/Tile program for the given schedule."""
    nc = bacc.Bacc(None, target_bir_lowering=False, debug=True, num_devices=n_cores)

    TR = sched.table_rows
    ZR = sched.zrows
    NT = sched.nt
    NPC, NPCP = sched.npc, sched.npcp
    NODE_TILES = TR // P

    # ---- inputs -------------------------------------------------------------
    xT = nc.dram_tensor("xT", [P, 2, TR], dt.float8e4, kind="ExternalInput")
    W1r = nc.dram_tensor("W1r", [P, 2, 512], dt.float8e4, kind="ExternalInput")
    W1Tr = nc.dram_tensor("W1Tr", [P, 4, 256], dt.bfloat16, kind="ExternalInput")
    Acat = nc.dram_tensor("Acat", [P, 4, 16], dt.bfloat16, kind="ExternalInput")
    W2r = nc.dram_tensor("W2r", [P, 4, 16], dt.bfloat16, kind="ExternalInput")
    W2Tr = nc.dram_tensor("W2Tr", [16, 512], dt.bfloat16, kind="ExternalInput")
    att2 = nc.dram_tensor("att2", [16, 2], dt.bfloat16, kind="ExternalInput")
    idx1A = nc.dram_tensor("idx1A", [P, sched.totA_cols], dt.int16, kind="ExternalInput")
    idx1B = nc.dram_tensor("idx1B", [P, sched.totB_cols], dt.int16, kind="ExternalInput")
    idx2A = nc.dram_tensor("idx2A", [P, sched.totA_cols], dt.int16, kind="ExternalInput")
    idx2B = nc.dram_tensor("idx2B", [P, sched.totB_cols], dt.int16, kind="ExternalInput")
    dstlocr = nc.dram_tensor("dstloc", [P, sched.nblocks], dt.float32, kind="ExternalInput")
    dstlocTr = nc.dram_tensor("dstlocT", [P, sched.nblocks, P], dt.bfloat16, kind="ExternalInput")
    out_shard = nc.dram_tensor("out_shard", [NPCP, 16], dt.float32, kind="ExternalOutput")

    with tile.TileContext(nc) as tc:
        nc.gpsimd.load_library(mlp)
        with (
            tc.tile_pool(name="dram", bufs=1, space="DRAM") as dram,
            tc.tile_pool(name="const", bufs=1) as cpool,
        ):
            table1 = dram.tile([TR, ROW1], dt.bfloat16)
            adst_own = dram.tile([NPCP, 8], dt.bfloat16)
            cc_in = dram.tile([NPCP, ROWZ], dt.bfloat16)
            cc_out = dram.tile([ZR, ROWZ], dt.bfloat16,
                               addr_space=("Shared" if n_cores > 1 else "Local"))

            # ---- constants -------------------------------------------------
            iota_i = cpool.tile([P, P], dt.int32)
            nc.gpsimd.iota(iota_i[:], pattern=[[1, P]], base=0, channel_multiplier=0)
            iota_bf = cpool.tile([P, P], dt.bfloat16)
            nc.vector.tensor_copy(iota_bf[:], iota_i[:])
            iota_ci = cpool.tile([P, 1], dt.int32)
            nc.gpsimd.iota(iota_ci[:], pattern=[[0, 1]], base=0, channel_multiplier=1)
            iota_cf = cpool.tile([P, 1], dt.float32)
            nc.vector.tensor_copy(iota_cf[:], iota_ci[:])
            ident = cpool.tile([P, P], dt.bfloat16)
            make_identity(nc, ident[:])

            W1s = cpool.tile([P, 2, 512], dt.float8e4)
            nc.sync.dma_start(W1s[:], W1r[:])
            W1Ts = cpool.tile([P, 4, 256], dt.bfloat16)
            nc.sync.dma_start(W1Ts[:], W1Tr[:])
            Acats = cpool.tile([P, 4, 16], dt.bfloat16)
            nc.sync.dma_start(Acats[:], Acat[:])
            W2Ts = cpool.tile([16, 512], dt.bfloat16)
            nc.sync.dma_start(W2Ts[:], W2Tr[:])
            att2s = cpool.tile([16, 2], dt.bfloat16)
            nc.sync.dma_start(att2s[:], att2[:])
            dstloc_s = cpool.tile([P, sched.nblocks], dt.float32)
            nc.sync.dma_start(dstloc_s[:], dstlocr[:])
            i1A = cpool.tile([P, sched.totA_cols], dt.int16)
            nc.sync.dma_start(i1A[:], idx1A[:])
            i1B = cpool.tile([P, sched.totB_cols], dt.int16)
            nc.sync.dma_start(i1B[:], idx1B[:])
            i2A = cpool.tile([P, sched.totA_cols], dt.int16)
            nc.sync.dma_start(i2A[:], idx2A[:])
            i2B = cpool.tile([P, sched.totB_cols], dt.int16)
            nc.sync.dma_start(i2B[:], idx2B[:])
            adst_all = cpool.tile([P, NT, 8], dt.bfloat16)
            azdst_all = cpool.tile([P, NT, 1], dt.bfloat16)

            # fused attention weights: wcat = W1 @ [Asrc|Adst]  -> [2,128,16]
            wcat = cpool.tile([P, 2, 16], dt.bfloat16)
            wcatq = cpool.tile([P, 2, 16], dt.float8e4)
            wz = cpool.tile([P, 4, 18], dt.bfloat16)   # [W2 | wcat2] per 128-chunk
            W2s = cpool.tile([P, 4, 16], dt.bfloat16)
            nc.sync.dma_start(W2s[:], W2r[:])
            with tc.tile_pool(name="p0ps", bufs=2, space="PSUM") as p0ps:
                for i in range(2):
                    ps = p0ps.tile([P, 16], dt.float32, tag="wc")
                    for j in range(4):
                        nc.tensor.matmul(
                            ps[:], W1Ts[:, j, i * P:(i + 1) * P], Acats[:, j, :],
                            start=(j == 0), stop=(j == 3),
                        )
                    nc.vector.tensor_copy(wcat[:, i, :], ps[:])
                    nc.vector.tensor_scalar(
                        out=wcatq[:, i, :], in0=ps[:], scalar1=16.0,
                        scalar2=None, op0=ALU.mult,
                    )
                for cch in range(4):
                    ps2 = p0ps.tile([P, 2], dt.float32, tag="wz")
                    nc.tensor.matmul(
                        ps2[:], W2Ts[:, cch * P:(cch + 1) * P], att2s[:],
                        start=True, stop=True,
                    )
                    nc.vector.tensor_copy(wz[:, cch, 16:18], ps2[:])
                    nc.vector.tensor_copy(wz[:, cch, 0:16], W2s[:, cch, :])

            # ---- P1: replicated node phase --------------------------------
            XB = 4  # node tiles per x load
            with (
                tc.tile_pool(name="p1sb", bufs=3) as p1sb,
                tc.tile_pool(name="p1ps", bufs=2, space="PSUM") as p1ps,
            ):
                DR = mybir.MatmulPerfMode.DoubleRow
                for tq in range(0, NODE_TILES, XB):
                    nb = min(XB, NODE_TILES - tq)
                    xt = p1sb.tile([P, 2, nb * P], dt.float8e4, tag="xt")
                    nc.sync.dma_start(xt[:], xT[:, :, tq * P: tq * P + nb * P])
                    for u in range(nb):
                        ph = p1ps.tile([P, 512], dt.float32, tag="ph")
                        pa = p1ps.tile([P, 16], dt.float32, tag="pa")
                        lhs = xt[:, :, u * P:(u + 1) * P]
                        nc.tensor.matmul(ph[:], lhs, W1s[:], perf_mode=DR,
                                         start=True, stop=True)
                        nc.tensor.matmul(pa[:], lhs, wcatq[:], perf_mode=DR,
                                         start=True, stop=True)
                        rowt = p1sb.tile([P, ROW1], dt.bfloat16, tag="rowt")
                        if u % 2 == 0:
                            nc.scalar.activation(rowt[:, 0:512], ph[:], AF.Copy,
                                                 scale=0.0625)
                        else:
                            nc.vector.tensor_scalar(
                                out=rowt[:, 0:512], in0=ph[:], scalar1=0.0625,
                                scalar2=None, op0=ALU.mult,
                            )
                        nc.vector.tensor_scalar(
                            out=rowt[:, 512:528], in0=pa[:], scalar1=0.0625,
                            scalar2=None, op0=ALU.mult,
                        )
                        nc.sync.dma_start(
                            table1[(tq + u) * P:(tq + u + 1) * P, :], rowt[:]
                        )

            # ---- P1.5: per-core a_dst table (SBUF, tile-major) ------------
            if phase not in ("p1",):
                rbase = nc.sync.partition_id() * NPC
                nc.sync.dma_start(
                    adst_own[:, :],
                    table1[bass.ds(rbase, NPCP), 520:528],
                )
                nc.sync.dma_start(
                    adst_all[:],
                    adst_own[:].rearrange("(t p) c -> p t c", p=P),
                )

            # ---- edge phase helper ----------------------------------------
            def edge_phase(layer):
                """layer 1: table1 gathers, 8 heads; layer 2: z-table, 1 head."""
                sub = os.environ.get("GAT_L1SUB", "full")
                if layer == 1:
                    g_src_tab_A, g_src_tab_B = table1, table1[sched.split1:, :]
                    g_elem, g_row = ROW1, ROW1
                    iA, iB = i1A, i1B
                    NH = 8
                    adst_t = adst_all
                else:
                    g_src_tab_A, g_src_tab_B = cc_out, cc_out[sched.zsplit:, :]
                    g_elem, g_row = ROWZ, ROWZ
                    iA, iB = i2A, i2B
                    NH = 1
                    adst_t = azdst_all
                MAXTB = sched.maxtb
                with (
                    tc.tile_pool(name=f"ed{layer}", bufs=2) as ep,
                    tc.tile_pool(name=f"sl{layer}", bufs=2) as sp,
                    tc.tile_pool(name=f"ms{layer}", bufs=2) as mp,
                    tc.tile_pool(name=f"eb{layer}", bufs=3) as bp,
                    tc.tile_pool(name=f"ep{layer}", bufs=2, space="PSUM") as pp,
                    tc.tile_pool(name=f"eo{layer}", bufs=1, space="PSUM") as po,
                    tc.tile_pool(name=f"eq{layer}", bufs=2, space="PSUM") as pq,
                ):
                    for g, tl in enumerate(sched.groups):
                        GB = sched.g_tblk[g]
                        GA = sched.g_ablk[g]
                        nA, nB_ = GA * P, (GB - GA) * P
                        gb = sched.g_base[g]
                        hg = ep.tile([P, GB, g_elem], dt.bfloat16, tag="hg")
                        if nA:
                            nc.gpsimd.dma_gather(
                                hg[:, 0:GA, :], g_src_tab_A[:],
                                iA[:, sched.gA_coloff[g]: sched.gA_coloff[g] + nA // 16],
                                nA, nA, g_elem, single_packet=False,
                            )
                        if nB_:
                            nc.gpsimd.dma_gather(
                                hg[:, GA:GB, :], g_src_tab_B,
                                iB[:, sched.gB_coloff[g]: sched.gB_coloff[g] + nB_ // 16],
                                nB_, nB_, g_elem, elem_step=g_row,
                                single_packet=False,
                            )
                        # transposed dstloc rows for this group (HWDGE stream)
                        dlT = ep.tile([P, GB, P], dt.bfloat16, tag="dlT")
                        nc.sync.dma_start(dlT[:], dstlocTr[:, gb:gb + GB, :])
                        if sub == "gather":
                            continue
                        for ti in tl:
                            ab, bb = int(sched.Ablk[ti]), int(sched.Bblk[ti])
                            tb = ab + bb
                            if tb == 0:
                                continue
                            apos, bpos = sched.tile_apos[ti], sched.tile_bpos[ti]
                            # (in-buffer block, w-slot) pairs for this tile
                            spans = []
                            if ab:
                                spans.append((apos, ab, 0))
                            if bb:
                                spans.append((bpos, bb, ab))
                            # selection matrices (both orientations, DVE only,
                            # one batched op per span) + a_dst broadcast:
                            # aDs[e, j, h] = a_dst[dstloc[e], h]
                            selS = sp.tile([P, MAXTB, P], dt.bfloat16, tag="selS")
                            sel2S = sp.tile([P, MAXTB, P], dt.bfloat16, tag="sel2S")
                            aDs = pq.tile([P, MAXTB, NH], dt.float32, tag="aDs")
                            for (o, n, wo) in spans:
                                gcol = gb + o
                                nc.vector.tensor_tensor(
                                    out=selS[:, wo:wo + n, :],
                                    in0=iota_bf[:].rearrange(
                                        "p (a b) -> p a b", a=1
                                    ).to_broadcast([P, n, P]),
                                    in1=dstloc_s[:, gcol:gcol + n].rearrange(
                                        "p (a b) -> p a b", b=1
                                    ).to_broadcast([P, n, P]),
                                    op=ALU.is_equal,
                                )
                                nc.vector.tensor_scalar(
                                    out=sel2S[:, wo:wo + n, :],
                                    in0=dlT[:, o:o + n, :],
                                    scalar1=iota_cf[:, 0:1],
                                    scalar2=None, op0=ALU.is_equal,
                                )
                                for k in range(n):
                                    j = wo + k
                                    nc.tensor.matmul(
                                        aDs[:, j, :], sel2S[:, j, :], adst_t[:, ti, :],
                                        start=True, stop=True,
                                    )
                            # scores: wbf (bf16) for L1 matmuls, wt (f32) for L2
                            if layer == 1:
                                wbf = bp.tile([P, tb * NH], dt.bfloat16, tag="wbf")
                            else:
                                wt = bp.tile([P, tb * NH], dt.float32, tag="wt")
                            for (o, n, wo) in spans:
                                st = bp.tile([P, n * NH], dt.float32, tag="st")
                                if layer == 1:
                                    a_s = hg[:, o:o + n, 512:520]
                                else:
                                    a_s = hg[:, o:o + n, 17:18]
                                st3 = st[:].rearrange("p (n k) -> p n k", k=NH)
                                nc.vector.tensor_tensor(
                                    out=st3, in0=a_s, in1=aDs[:, wo:wo + n, :], op=ALU.add
                                )
                                lk = bp.tile([P, n * NH], dt.float32, tag="lk")
                                nc.vector.scalar_tensor_tensor(
                                    lk[:], st[:], 0.2, st[:], ALU.mult, ALU.max
                                )
                                if layer == 1:
                                    nc.scalar.activation(
                                        wbf[:, wo * NH:(wo + n) * NH], lk[:], AF.Exp
                                    )
                                else:
                                    nc.scalar.activation(
                                        wt[:, wo * NH:(wo + n) * NH], lk[:], AF.Exp
                                    )
                            if sub == "score":
                                continue
                            if layer == 1:
                                psum_o = pp.tile([P, 512], dt.float32, tag="psO")
                                psum_d = po.tile([P, 8], dt.float32, tag="psD")
                            else:
                                psum_o = pp.tile([P, 17], dt.float32, tag="psO")
                                psum_d = None
                            j = 0
                            for (o, n, wo) in spans:
                                Ms = mp.tile(
                                    [P, MAXTB, 512 if layer == 1 else 17],
                                    dt.bfloat16, tag="Ms",
                                )
                                if layer == 1:
                                    wbc = wbf[:, wo * 8:(wo + n) * 8].rearrange(
                                        "p (n a b) -> p n a b", n=n, b=1
                                    ).to_broadcast([P, n, 8, 64])
                                    nc.vector.tensor_tensor(
                                        out=Ms[:, 0:n, :].rearrange(
                                            "p n (a b) -> p n a b", b=64
                                        ),
                                        in0=hg[:, o:o + n, 0:512].rearrange(
                                            "p n (a b) -> p n a b", b=64
                                        ),
                                        in1=wbc,
                                        op=ALU.mult,
                                    )
                                else:
                                    w2bc = wt[:, wo:wo + n].rearrange(
                                        "p (n b) -> p n b", b=1
                                    ).to_broadcast([P, n, 17])
                                    nc.vector.tensor_tensor(
                                        out=Ms[:, 0:n, :], in0=hg[:, o:o + n, 0:17],
                                        in1=w2bc, op=ALU.mult,
                                    )
                                for k in range(n):
                                    jj = wo + k
                                    first, last = (j == 0), (j == tb - 1)
                                    nc.tensor.matmul(psum_o[:], selS[:, jj, :], Ms[:, k, :], start=first, stop=last)
                                    if layer == 1:
                                        nc.tensor.matmul(
                                            psum_d[:], selS[:, jj, :], wbf[:, jj * 8:jj * 8 + 8],
                                            start=first, stop=last,
                                        )
                                    j += 1
                            if sub == "blocks":
                                continue
                            close_tile(layer, ti, psum_o, psum_d, bp, pp, po)

            # ---- tile close -------------------------------------------------
            def close_tile(layer, ti, psum_o, psum_d, bp, pp, pq):
                if layer == 1:
                    r = bp.tile([P, 8], dt.float32, tag="r")
                    nc.vector.reciprocal(r[:], psum_d[:])
                    o1 = bp.tile([P, 512], dt.bfloat16, tag="o1")
                    o13 = o1[:].rearrange("p (a b) -> p a b", b=64)
                    rbc = r[:].rearrange("p (a b) -> p a b", b=1).to_broadcast([P, 8, 64])
                    nc.vector.tensor_tensor(
                        out=o13,
                        in0=psum_o[:].rearrange("p (a b) -> p a b", b=64),
                        in1=rbc, op=ALU.mult,
                    )
                    # elu: h2 = max(o1,0) + exp(min(o1,0)) - 1
                    u = bp.tile([P, 512], dt.bfloat16, tag="u")
                    nc.vector.tensor_scalar_min(u[:], o1[:], 0.0)
                    e1 = bp.tile([P, 512], dt.bfloat16, tag="e1")
                    nc.scalar.activation(e1[:], u[:], AF.Exp)
                    rv = bp.tile([P, 512], dt.bfloat16, tag="rv")
                    nc.vector.tensor_scalar_max(rv[:], o1[:], 0.0)
                    h2 = bp.tile([P, 512], dt.bfloat16, tag="h2")
                    nc.vector.scalar_tensor_tensor(
                        h2[:], e1[:], -1.0, rv[:], ALU.add, ALU.add
                    )
                    # transpose h2 -> z matmuls
                    pz = pq.tile([P, 18], dt.float32, tag="psZ")
                    for c in range(4):
                        ptr = pp.tile([P, P], dt.bfloat16, tag="psT")
                        nc.tensor.transpose(ptr[:], h2[:, c * P:(c + 1) * P], ident[:])
                        h2T = bp.tile([P, P], dt.bfloat16, tag="h2T")
                        nc.scalar.copy(h2T[:], ptr[:])
                        nc.tensor.matmul(pz[:], h2T[:], wz[:, c, :], start=(c == 0), stop=(c == 3))
                    # z-row layout: [z 16 | 1.0 | a_src2 | a_dst2 | junk]
                    zrow = bp.tile([P, ROWZ], dt.bfloat16, tag="zrow")
                    nc.vector.tensor_copy(zrow[:, 0:16], pz[:, 0:16])
                    nc.vector.memset(zrow[:, 16:17], 1.0)
                    nc.vector.tensor_copy(zrow[:, 17:19], pz[:, 16:18])
                    nc.sync.dma_start(cc_in[ti * P:(ti + 1) * P, :], zrow[:])
                else:
                    r2 = bp.tile([P, 1], dt.float32, tag="r2")
                    nc.vector.reciprocal(r2[:], psum_o[:, 16:17])
                    o2 = bp.tile([P, 16], dt.float32, tag="o2")
                    nc.vector.tensor_scalar_mul(o2[:], psum_o[:, 0:16], r2[:, 0:1])
                    mx = bp.tile([P, 1], dt.float32, tag="mx")
                    nc.vector.tensor_reduce(mx[:], o2[:], axis=mybir.AxisListType.X, op=ALU.max)
                    o2m = bp.tile([P, 16], dt.float32, tag="o2m")
                    nc.vector.tensor_scalar_sub(o2m[:], o2[:], mx[:, 0:1])
                    ex = bp.tile([P, 16], dt.float32, tag="ex")
                    ssum = bp.tile([P, 1], dt.float32, tag="ssum")
                    nc.scalar.activation(ex[:], o2m[:], AF.Exp, accum_out=ssum[:])
                    lse = bp.tile([P, 1], dt.float32, tag="lse")
                    nc.scalar.activation(lse[:], ssum[:], AF.Ln)
                    res = bp.tile([P, 16], dt.float32, tag="res")
                    nc.vector.tensor_scalar_sub(res[:], o2m[:], lse[:, 0:1])
                    nc.sync.dma_start(out_shard[ti * P:(ti + 1) * P, :], res[:])

            if phase not in ("p1", "p15"):
                edge_phase(1)

            if phase in ("cc", "full"):
                # ---- z-table exchange -------------------------------------
                if n_cores == 1:
                    nc.sync.dma_start(cc_out[:, :], cc_in[:, :])
                else:
                    nc.gpsimd.collective_compute(
                        "AllGather", ALU.bypass,
                        ins=[cc_in[:]], outs=[cc_out[:]],
                        replica_groups=[list(range(n_cores))],
                    )

            if phase == "full":
                nc.sync.dma_start(
                    azdst_all[:],
                    cc_in[:, 18:19].rearrange("(t p) c -> p t c", p=P),
                )
                edge_phase(2)

    nc.compile()
    return nc


# ----------------------------------------------------------------------------
# host entry
# ----------------------------------------------------------------------------

def _blockdiag(att, heads, hid):
    """[heads, hid] -> [heads*hid, heads] block diagonal."""
    out = np.zeros((heads * hid, max(heads, 1)), np.float32)
    for h in range(heads):
        out[h * hid:(h + 1) * hid, h] = att[h]
    return out


def prepare_inputs(inputs, sched: Schedule):
    x = np.asarray(inputs["x"], np.float32)
    ei = np.asarray(inputs["edge_index"])
    W1 = np.asarray(inputs["W1"], np.float32)
    as1 = np.asarray(inputs["att_src1"], np.float32)
    ad1 = np.asarray(inputs["att_dst1"], np.float32)
    W2 = np.asarray(inputs["W2"], np.float32)
    as2 = np.asarray(inputs["att_src2"], np.float32)
    ad2 = np.asarray(inputs["att_dst2"], np.float32)

    N, IN = x.shape
    TR = sched.table_rows
    xp = np.zeros((TR, IN), np.float32)
    xp[:N] = x
    F8 = ml_dtypes.float8_e4m3
    xTb = np.ascontiguousarray(
        xp.T.reshape(2, P, TR).transpose(1, 0, 2)).astype(F8)
    W1b = np.ascontiguousarray(
        (W1 * 16.0).reshape(2, P, 512).transpose(1, 0, 2)).astype(F8)
    W1Tb = np.ascontiguousarray(
        W1.T.reshape(4, P, 256).transpose(1, 0, 2)).astype(BF)
    AcatB = np.ascontiguousarray(
        np.concatenate([_blockdiag(as1, 8, 64), _blockdiag(ad1, 8, 64)], axis=1)
        .reshape(4, P, 16).transpose(1, 0, 2)
    ).astype(BF)
    W2b = np.ascontiguousarray(W2.reshape(4, P, 16).transpose(1, 0, 2)).astype(BF)
    W2Tb = np.ascontiguousarray(W2.T).astype(BF)          # [16, 512]
    att2b = np.concatenate([as2.T, ad2.T], axis=1).astype(BF)  # [16, 2]

    shared = dict(xT=xTb, W1r=W1b, W1Tr=W1Tb, Acat=AcatB, W2r=W2b, W2Tr=W2Tb, att2=att2b)
    maps = []
    for c in range(sched.n_cores):
        pc = sched.per_core[c]
        m = dict(shared)
        m.update(
            idx1A=pc["idx1A"], idx1B=pc["idx1B"], idx2A=pc["idx2A"],
            idx2B=pc["idx2B"], dstloc=pc["dstloc"], dstlocT=pc["dstlocT"],
        )
        maps.append(m)
    return maps


_LAST_RESULT = {}


def kernel(**inputs):
    from concourse.bass_utils import run_bass_kernel_spmd

    x = np.asarray(inputs["x"], np.float32)
    ei = np.asarray(inputs["edge_index"], np.int64)
    N = x.shape[0]
    n_cores = 8
    loops = np.arange(N, dtype=np.int64)
    src = np.concatenate([ei[0], loops])
    dst = np.concatenate([ei[1], loops])

    sched = Schedule(src, dst, N, n_cores)
    phase = os.environ.get("GAT_PHASE", "full")
    nc = build_program(sched, n_cores, phase=phase)
    in_maps = prepare_inputs(inputs, sched)

    trace = bool(int(os.environ.get("GAT_TRACE", "0")))
    res = run_bass_kernel_spmd(
        nc, in_maps, core_ids=list(range(n_cores)), trace=trace,
    )
    _LAST_RESULT["res"] = res

    out = np.zeros((N, 16), np.float32)
    for c in range(n_cores):
        sh = res.results[c]["out_shard"]
        n0 = c * sched.npc
        out[n0:n0 + sched.npc] = sh[: sched.npc]
    return out

